# revision 9
# baseline (speedup 1.0000x reference)
"""Trainium2 Bass kernel for nn_EnsemblesWithMessagePassing.

Strategy: data-parallel over token positions (shard N=512 across the 8
NeuronCores, 64 positions each => 128 (b,n) tokens per core). The voting
attention is strictly per-position over the M=16 local messages, so this
sharding needs no collectives: each core holds every ensemble member's
weights and computes all 8 members for its position slice.

On-chip dataflow per core (feature-major activations for the PE):
  A) rms stats of tokens via DVE square + ones-column matmul (cross-
     partition sum), rsqrt; a [1,T]-lhsT rank-1 matmul moves the scale to
     token-major [T,1] so it fuses into the q/gates PSUM->SBUF copy.
  B) q = (x @ wq') * rs  and  gates = sigmoid((x @ wg') * rs); norm_w,
     knorm_w and the attention scale are pre-folded into wq'/wg' on CPU.
  C) per member l: kv for the token message; member Linear (token-major,
     bias added via a K=1 rank-1 matmul into the same PSUM accumulation
     group); PE-transpose of the Linear output; kv for the output message.
     k stays RAW -- its rms scale (dh-independent) is folded into sim
     later, so the sim products for a message are emitted immediately
     after its kv and overlap the remaining members' PE work.
  E) per member l: sim *= krinv, softmax over m (max-subtracted), gate
     folded into attn, o = reduce over m of attn*v, then PE-transpose of
     o and pooled = o @ wout, DMA straight from PSUM out.

Matmuls run as float32r (TF32-style datapath, fp32 PSUM accumulation);
everything else is fp32.
"""
import sys

for _p in ("/opt/trn_rl_repo", "/root/.axon_site/_ro/trn_rl_repo"):
    if _p not in sys.path:
        sys.path.insert(0, _p)

try:  # NTFF profile hook glue (only needed if BASS_TRACE is set externally)
    import antenv.axon_hooks  # noqa: F401
except Exception:
    try:
        import importlib.util as _ilu
        import antenv as _antenv

        _spec = _ilu.spec_from_file_location(
            "antenv.axon_hooks", "/opt/trn_rl_repo/antenv/axon_hooks.py")
        _mod = _ilu.module_from_spec(_spec)
        _spec.loader.exec_module(_mod)
        _antenv.axon_hooks = _mod
        sys.modules["antenv.axon_hooks"] = _mod
    except Exception:
        pass

from contextlib import ExitStack

import numpy as np

import concourse.bass as bass
import concourse.tile as tile
from concourse import bacc, mybir
from concourse import bass_utils
from concourse.masks import make_identity

f32 = mybir.dt.float32
f32r = mybir.dt.float32r
bf16 = mybir.dt.bfloat16
AF = mybir.ActivationFunctionType
AL = mybir.AluOpType
AX = mybir.AxisListType

# problem shape
L, B, N, D = 8, 2, 512, 1024
H, DH = 8, 64
INNER = H * DH          # 512
M = 2 * L               # 16 messages
SCALE = DH ** -0.5
EPS = float(np.finfo(np.float32).eps)

NCORES = 8
NSL = N // NCORES       # 64 positions per core per batch row
T = B * NSL             # 128 tokens per core
LT = L * T              # 1024
DT = D // 128           # 8 d-tiles
IT = INNER // 128       # 4 inner-tiles

# compute dtype config
MM_DT = "float32r"      # matmul operand dtype: "float32r" | "float32"
ATT_DT = "bfloat16"     # attention DVE dtype: "float32" | "bfloat16"

_NC_CACHE = {}


def _build(mm_dt_name=MM_DT, att_dt_name=ATT_DT):
    mdt = getattr(mybir.dt, mm_dt_name)
    adt = getattr(mybir.dt, att_dt_name)

    nc = bacc.Bacc("TRN2", target_bir_lowering=False, debug=False,
                   enable_asserts=False, num_devices=NCORES)

    xT_d = nc.dram_tensor("xT", [128, DT, LT], mdt, kind="ExternalInput").ap()
    wnet_d = nc.dram_tensor("wnetT", [L, 128, DT, D], mdt, kind="ExternalInput").ap()
    wq_d = nc.dram_tensor("wqT", [128, DT, INNER], mdt, kind="ExternalInput").ap()
    wkv_d = nc.dram_tensor("wkvT", [128, DT, 2 * INNER], mdt, kind="ExternalInput").ap()
    wg_d = nc.dram_tensor("wgT", [128, DT, H], mdt, kind="ExternalInput").ap()
    wout_d = nc.dram_tensor("woutT", [128, IT, D], mdt, kind="ExternalInput").ap()
    bnet_d = nc.dram_tensor("bnet", [1, L, D], mdt, kind="ExternalInput").ap()
    onesc_d = nc.dram_tensor("onesc", [128, 128], mdt, kind="ExternalInput").ap()
    out_d = nc.dram_tensor("out", [L, T, D], f32, kind="ExternalOutput").ap()

    def asf32(ap):
        # DVE/ACT read float32r tiles as plain fp32 bits
        return ap.bitcast(f32) if ap.dtype == f32r else ap

    with tile.TileContext(nc) as tc, ExitStack() as ctx:
        pc = ctx.enter_context(tc.tile_pool(name="const", bufs=1))
        pa = ctx.enter_context(tc.tile_pool(name="attp", bufs=1))
        pmm = ctx.enter_context(tc.tile_pool(name="psmm", bufs=3, space="PSUM"))
        ptp = ctx.enter_context(tc.tile_pool(name="pstp", bufs=2, space="PSUM"))
        psm = ctx.enter_context(tc.tile_pool(name="pssm", bufs=2, space="PSUM"))

        # ---- constants ----
        ident = pc.tile([128, 128], f32, tag="ident")
        make_identity(nc, ident)
        ones_m = pc.tile([128, 128], mdt, tag="ones_m")          # matmul-dtype ones
        nc.sync.dma_start(ones_m[:], onesc_d[:])
        ones_f = pc.tile([1, 2], f32, tag="ones_f")              # fp32 ones
        nc.vector.memset(ones_f[:], 1.0)
        eps_c = pc.tile([128, 1], f32, tag="eps")
        nc.vector.memset(eps_c[:], EPS)

        # whole-kernel attention state
        k_all = pa.tile([128, M, INNER], adt, tag="k_all")   # RAW k
        v_all = pa.tile([128, M, INNER], adt, tag="v_all")
        g_all = pa.tile([128, L, H], f32, tag="g_all")
        kss_all = pa.tile([128, M, H], f32, tag="kss_all")
        sim_all = pa.tile([128, L, H, M], f32, tag="sim_all")
        krms = pa.tile([128, M, H], f32, tag="krms")
        krinv = pa.tile([128, M, H], f32, tag="krinv")

        with ExitStack() as ctx_b:
            pb = ctx_b.enter_context(tc.tile_pool(name="bigact", bufs=1))
            ps_ = ctx_b.enter_context(tc.tile_pool(name="scr", bufs=2))

            xT = pb.tile([128, DT, LT], mdt, tag="xT")
            nc.sync.dma_start(xT[:], xT_d[:])
            wkv = pb.tile([128, DT, 2 * INNER], mdt, tag="wkv")
            nc.sync.dma_start(wkv[:], wkv_d[:])
            q_all = pb.tile([128, L, INNER], adt, tag="q_all")

            with ExitStack() as ctx_w:
                pw = ctx_w.enter_context(tc.tile_pool(name="qgp", bufs=1))
                wq = pw.tile([128, DT, INNER], mdt, tag="wq")
                nc.sync.dma_start(wq[:], wq_d[:])
                wg = pw.tile([128, DT, H], mdt, tag="wg")
                nc.sync.dma_start(wg[:], wg_d[:])

                # ---- phase A: token rms stats ----
                with nc.named_scope("stats"):
                    rms_row = pw.tile([1, LT], f32, tag="rms_row")
                    for c in range(2):
                        ssps = psm.tile([2, 512], f32, tag="sm")
                        for d in range(DT):
                            sq = ps_.tile([128, 512], mdt, tag="sq")
                            nc.vector.tensor_tensor(
                                sq[:], asf32(xT[:, d, c * 512:(c + 1) * 512]),
                                asf32(xT[:, d, c * 512:(c + 1) * 512]), AL.mult)
                            nc.tensor.matmul(ssps[:], ones_m[:, 0:2], sq[:],
                                             start=(d == 0), stop=(d == DT - 1))
                        nc.scalar.activation(rms_row[0:1, c * 512:(c + 1) * 512],
                                             ssps[0:1, :], AF.Sqrt,
                                             scale=1.0 / D, bias=eps_c[0:1, 0:1])
                    rs_row = pw.tile([1, LT], f32, tag="rs_row")
                    nc.vector.reciprocal(rs_row[:], rms_row[:])
                    # move scale to token-major: rs_tok[t, l]
                    rsps = psm.tile([128, L], f32, tag="sm")
                    for l in range(L):
                        nc.tensor.matmul(rsps[:, l:l + 1],
                                         rs_row[0:1, l * T:(l + 1) * T],
                                         ones_f[0:1, 0:1], start=True, stop=True)
                    rs_tok = pw.tile([128, L], f32, tag="rs_tok")
                    nc.vector.tensor_copy(rs_tok[:], rsps[:])

                # ---- phase D: queries and gates ----
                with nc.named_scope("qg"):
                    for l in range(L):
                        qps = pmm.tile([128, INNER], f32, tag="mm")
                        for d in range(DT):
                            nc.tensor.matmul(qps[:], xT[:, d, l * T:(l + 1) * T],
                                             wq[:, d], start=(d == 0),
                                             stop=(d == DT - 1))
                        nc.scalar.activation(q_all[:, l], qps[:], AF.Copy,
                                             scale=rs_tok[:, l:l + 1])
                    for l in range(L):
                        gps = psm.tile([128, H], f32, tag="sm")
                        for d in range(DT):
                            nc.tensor.matmul(gps[:], xT[:, d, l * T:(l + 1) * T],
                                             wg[:, d], start=(d == 0),
                                             stop=(d == DT - 1))
                        nc.scalar.activation(g_all[:, l], gps[:], AF.Sigmoid,
                                             scale=rs_tok[:, l:l + 1])

            # ---- phase C: member Linear + kv + sim products ----
            def kv_msg(lhsT_of_dt, m):
                for half in range(2):  # 0 = k, 1 = v
                    ps = pmm.tile([128, INNER], f32, tag="mm")
                    for d in range(DT):
                        nc.tensor.matmul(ps[:], lhsT_of_dt(d),
                                         wkv[:, d, half * INNER:(half + 1) * INNER],
                                         start=(d == 0), stop=(d == DT - 1))
                    if half == 0:
                        nc.scalar.copy(k_all[:, m], ps[:])
                        ksq = ps_.tile([128, INNER], f32, tag="ksq")
                        nc.gpsimd.tensor_tensor(ksq[:], asf32(k_all[:, m]),
                                                asf32(k_all[:, m]), AL.mult)
                        nc.vector.tensor_reduce(
                            kss_all[:, m], ksq.rearrange("p (h d) -> p h d", d=DH),
                            axis=AX.X, op=AL.add)
                    else:
                        nc.scalar.copy(v_all[:, m], ps[:])

            def sim_msg(m):
                for lq in range(L):
                    scr = ps_.tile([128, INNER], adt, tag="scr")
                    nc.vector.tensor_tensor(scr[:], asf32(q_all[:, lq]),
                                            asf32(k_all[:, m]), AL.mult)
                    nc.vector.tensor_reduce(
                        sim_all[:, lq, :, m],
                        scr.rearrange("p (h d) -> p h d", d=DH),
                        axis=AX.X, op=AL.add)

            with ExitStack() as ctx_l, nc.named_scope("linkv"):
                plk = ctx_l.enter_context(tc.tile_pool(name="linp", bufs=1))
                plw = ctx_l.enter_context(tc.tile_pool(name="linw", bufs=2))
                for l in range(L):
                    kv_msg(lambda d: xT[:, d, l * T:(l + 1) * T], l)
                    sim_msg(l)
                    # member Linear, token-major out, in 4 chunks of 256 cols
                    bnet_l = plk.tile([1, D], mdt, tag="bnet_l")
                    nc.sync.dma_start(bnet_l[:], bnet_d[0:1, l])
                    osb = plk.tile([128, D], f32, tag="osb")
                    for oc in range(4):
                        wnt = plw.tile([128, DT, 256], mdt, tag="wnt")
                        nc.sync.dma_start(wnt[:],
                                          wnet_d[l][:, :, oc * 256:(oc + 1) * 256])
                        ps = pmm.tile([128, 256], f32, tag="mm")
                        for d in range(DT):
                            nc.tensor.matmul(ps[:], xT[:, d, l * T:(l + 1) * T],
                                             wnt[:, d], start=(d == 0), stop=False)
                        nc.tensor.matmul(ps[:], ones_m[0:1, :],
                                         bnet_l[0:1, oc * 256:(oc + 1) * 256],
                                         start=False, stop=True)
                        nc.scalar.copy(osb[:, oc * 256:(oc + 1) * 256], ps[:])
                    # transpose Linear output to feature-major
                    oT = plk.tile([128, DT, T], mdt, tag="oT")
                    for ot in range(DT):
                        tps = ptp.tile([128, 128], f32, tag="tp")
                        nc.tensor.transpose(tps[:], osb[:, ot * 128:(ot + 1) * 128],
                                            ident[:])
                        nc.scalar.copy(oT[:, ot], tps[:])
                    kv_msg(lambda d: oT[:, d], L + l)
                    sim_msg(L + l)

            # k rms scales (applied to sim, not k: dh-independent)
            nc.scalar.activation(krms.rearrange("p m h -> p (m h)"),
                                 kss_all.rearrange("p m h -> p (m h)"),
                                 AF.Sqrt, scale=1.0 / DH, bias=eps_c[:, 0:1])
            nc.vector.reciprocal(krinv.rearrange("p m h -> p (m h)"),
                                 krms.rearrange("p m h -> p (m h)"))

        # ---- phase E/F: softmax + o + pooled, per member ----
        with ExitStack() as ctx_e:
            pe = ctx_e.enter_context(tc.tile_pool(name="outp", bufs=1))
            pes = ctx_e.enter_context(tc.tile_pool(name="outs", bufs=2))
            wout = pe.tile([128, IT, D], mdt, tag="wout")
            nc.sync.dma_start(wout[:], wout_d[:])
            o_all = pe.tile([128, L, INNER], f32, tag="o_all")
            H2 = H // 2
            v_re = v_all.rearrange("p m (h d) -> p h d m", d=DH)
            krinv_re = krinv.rearrange("p m h -> p h m")
            with nc.named_scope("attn"):
                for l in range(L):
                    sim = sim_all[:, l]                       # [128, H, M]
                    nc.vector.tensor_tensor(sim, sim, krinv_re, AL.mult)
                    mx = pes.tile([128, H], f32, tag="mx")
                    nc.vector.tensor_reduce(mx[:], sim, axis=AX.X, op=AL.max)
                    nc.vector.tensor_tensor(
                        sim, sim, mx[:, :, None].to_broadcast([128, H, M]),
                        AL.subtract)
                    pl = pes.tile([128, H, M], adt, tag="pl")
                    nc.scalar.activation(pl[:], sim, AF.Exp)
                    sm = pes.tile([128, H], f32, tag="sme")
                    nc.vector.tensor_reduce(sm[:], asf32(pl[:]), axis=AX.X, op=AL.add)
                    rgf = pes.tile([128, H], f32, tag="rgf")
                    nc.vector.reciprocal(rgf[:], sm[:])
                    rg = pes.tile([128, H], adt, tag="rg")
                    nc.vector.tensor_tensor(rg[:], rgf[:], g_all[:, l], AL.mult)
                    nc.vector.tensor_tensor(
                        pl[:], pl[:], rg[:, :, None].to_broadcast([128, H, M]),
                        AL.mult)
                    o_l = o_all[:, l].rearrange("p (h d) -> p h d", d=DH)
                    for hh in range(2):
                        hs = slice(hh * H2, (hh + 1) * H2)
                        prod = pe.tile([128, H2, DH, M], adt, tag="prod")
                        nc.gpsimd.tensor_tensor(
                            prod[:], v_re[:, hs],
                            pl[:, hs, None, :].to_broadcast([128, H2, DH, M]),
                            AL.mult)
                        nc.vector.tensor_reduce(
                            o_l[:, hs], prod[:], axis=AX.X, op=AL.add)
                    # pooled = o @ wout
                    oTt = pes.tile([128, IT, T], mdt, tag="oTt")
                    for it in range(IT):
                        tps = ptp.tile([128, 128], f32, tag="tp")
                        nc.tensor.transpose(
                            tps[:], o_all[:, l, it * 128:(it + 1) * 128], ident[:])
                        nc.scalar.copy(oTt[:, it], tps[:])
                    pout = pes.tile([128, D], f32, tag="pout")
                    for oc in range(2):
                        ps = pmm.tile([128, 512], f32, tag="mm")
                        for it in range(IT):
                            nc.tensor.matmul(ps[:], oTt[:, it],
                                             wout[:, it, oc * 512:(oc + 1) * 512],
                                             start=(it == 0), stop=(it == IT - 1))
                        nc.scalar.copy(pout[:, oc * 512:(oc + 1) * 512], ps[:])
                    nc.sync.dma_start(out_d[l][:], pout[:])

    nc.compile()
    return nc


def get_nc():
    key = (MM_DT, ATT_DT)
    if key not in _NC_CACHE:
        _NC_CACHE[key] = _build(*key)
    return _NC_CACHE[key]


def prep_weights(w_net, b_net, norm_w, wq, wkv, knorm_w, wg, wout):
    """CPU-side layout prep shared by all cores (fp32 contiguous arrays)."""
    wnetT = np.ascontiguousarray(
        w_net.reshape(L, D, DT, 128).transpose(0, 3, 2, 1))          # [L,128,DT,D]
    colscale = (np.tile(knorm_w, H) * SCALE).astype(np.float32)
    wq2 = norm_w[:, None] * wq * colscale[None, :]
    wqT = np.ascontiguousarray(wq2.reshape(DT, 128, INNER).transpose(1, 0, 2))
    wkvT = np.ascontiguousarray(wkv.reshape(DT, 128, 2 * INNER).transpose(1, 0, 2))
    wgT = np.ascontiguousarray((norm_w[:, None] * wg).reshape(DT, 128, H)
                               .transpose(1, 0, 2))
    woutT = np.ascontiguousarray(wout.reshape(IT, 128, D).transpose(1, 0, 2))
    return dict(
        wnetT=wnetT.astype(np.float32),
        wqT=wqT.astype(np.float32),
        wkvT=wkvT.astype(np.float32),
        wgT=wgT.astype(np.float32),
        woutT=woutT.astype(np.float32),
        bnet=np.ascontiguousarray(b_net[None]).astype(np.float32),
        onesc=np.ones((128, 128), dtype=np.float32),
    )


def prep_core_x(tokens, c):
    """Per-core feature-major token slice: [128, DT, LT]."""
    xs = tokens[:, :, c * NSL:(c + 1) * NSL, :].reshape(L, T, D)
    xT = xs.reshape(L, T, DT, 128).transpose(3, 2, 0, 1).reshape(128, DT, LT)
    return np.ascontiguousarray(xT).astype(np.float32)


def make_in_maps(tokens, w_net, b_net, norm_w, wq, wkv, knorm_w, wg, wout):
    shared = prep_weights(np.asarray(w_net, np.float32), np.asarray(b_net, np.float32),
                          np.asarray(norm_w, np.float32), np.asarray(wq, np.float32),
                          np.asarray(wkv, np.float32), np.asarray(knorm_w, np.float32),
                          np.asarray(wg, np.float32), np.asarray(wout, np.float32))
    tokens = np.asarray(tokens, np.float32)
    return [dict(shared, xT=prep_core_x(tokens, c)) for c in range(NCORES)]


def stitch(results):
    full = np.empty((L, B, N, D), dtype=np.float32)
    for c in range(NCORES):
        full[:, :, c * NSL:(c + 1) * NSL, :] = \
            results[c]["out"].reshape(L, B, NSL, D)
    return full


def kernel(tokens, w_net, b_net, norm_w, wq, wkv, knorm_w, wg, wout):
    nc = get_nc()
    in_maps = make_in_maps(tokens, w_net, b_net, norm_w, wq, wkv, knorm_w, wg, wout)
    res = bass_utils.run_bass_kernel_spmd(nc, in_maps, core_ids=list(range(NCORES)))
    return stitch(res.results)


# revision 11
# speedup vs baseline: 1.2440x; 1.2440x over previous
"""Trainium2 Bass kernel for nn_EnsemblesWithMessagePassing.

Strategy: data-parallel over token positions (shard N=512 across the 8
NeuronCores, 64 positions each => 128 (b,n) tokens per core). The voting
attention is strictly per-position over the M=16 local messages, so this
sharding needs no collectives: each core holds every ensemble member's
weights and computes all 8 members for its position slice.

On-chip dataflow per core (feature-major activations for the PE):
  A) rms stats of tokens via DVE square + ones-column matmul (cross-
     partition sum), rsqrt; a [1,T]-lhsT rank-1 matmul moves the scale to
     token-major [T,1] so it fuses into the q/gates PSUM->SBUF copy.
  B) q = (x @ wq') * rs  and  gates = sigmoid((x @ wg') * rs); norm_w,
     knorm_w and the attention scale are pre-folded into wq'/wg' on CPU.
  C) per member l: kv for the token message; member Linear (token-major,
     bias added via a K=1 rank-1 matmul into the same PSUM accumulation
     group); PE-transpose of the Linear output; kv for the output message.
     k stays RAW -- its rms scale (dh-independent) is folded into sim
     later, so the sim products for a message are emitted immediately
     after its kv and overlap the remaining members' PE work.
  E) per member l: sim *= krinv, softmax over m (max-subtracted), gate
     folded into attn, o = reduce over m of attn*v, then PE-transpose of
     o and pooled = o @ wout, DMA straight from PSUM out.

Matmuls run as float32r (TF32-style datapath, fp32 PSUM accumulation);
everything else is fp32.
"""
import sys

for _p in ("/opt/trn_rl_repo", "/root/.axon_site/_ro/trn_rl_repo"):
    if _p not in sys.path:
        sys.path.insert(0, _p)

try:  # NTFF profile hook glue (only needed if BASS_TRACE is set externally)
    import antenv.axon_hooks  # noqa: F401
except Exception:
    try:
        import importlib.util as _ilu
        import antenv as _antenv

        _spec = _ilu.spec_from_file_location(
            "antenv.axon_hooks", "/opt/trn_rl_repo/antenv/axon_hooks.py")
        _mod = _ilu.module_from_spec(_spec)
        _spec.loader.exec_module(_mod)
        _antenv.axon_hooks = _mod
        sys.modules["antenv.axon_hooks"] = _mod
    except Exception:
        pass

from contextlib import ExitStack

import numpy as np

import concourse.bass as bass
import concourse.tile as tile
from concourse import bacc, mybir
from concourse import bass_utils
from concourse.masks import make_identity

f32 = mybir.dt.float32
f32r = mybir.dt.float32r
bf16 = mybir.dt.bfloat16
AF = mybir.ActivationFunctionType
AL = mybir.AluOpType
AX = mybir.AxisListType

# problem shape
L, B, N, D = 8, 2, 512, 1024
H, DH = 8, 64
INNER = H * DH          # 512
M = 2 * L               # 16 messages
SCALE = DH ** -0.5
EPS = float(np.finfo(np.float32).eps)

NCORES = 8
NSL = N // NCORES       # 64 positions per core per batch row
T = B * NSL             # 128 tokens per core
LT = L * T              # 1024
DT = D // 128           # 8 d-tiles
IT = INNER // 128       # 4 inner-tiles

# compute dtype config
MM_DT = "float32r"      # matmul operand dtype: "float32r" | "float32"
ATT_DT = "bfloat16"     # attention DVE dtype: "float32" | "bfloat16"

_NC_CACHE = {}


def _build(mm_dt_name=MM_DT, att_dt_name=ATT_DT):
    mdt = getattr(mybir.dt, mm_dt_name)   # light stages: stats, q, gates, pooled
    adt = getattr(mybir.dt, att_dt_name)  # attention elementwise dtype
    hdt = bf16                            # heavy stages: member Linear + kv

    nc = bacc.Bacc("TRN2", target_bir_lowering=False, debug=False,
                   enable_asserts=False, num_devices=NCORES)

    xT_d = nc.dram_tensor("xT", [128, DT, LT], mdt, kind="ExternalInput").ap()
    xTb_d = nc.dram_tensor("xTb", [128, DT, LT], hdt, kind="ExternalInput").ap()
    wnet_d = nc.dram_tensor("wnetT", [L, 128, DT, D], hdt, kind="ExternalInput").ap()
    wq_d = nc.dram_tensor("wqT", [128, DT, INNER], mdt, kind="ExternalInput").ap()
    wkv_d = nc.dram_tensor("wkvT", [128, DT, 2 * INNER], hdt, kind="ExternalInput").ap()
    wg_d = nc.dram_tensor("wgT", [128, DT, H], mdt, kind="ExternalInput").ap()
    wout_d = nc.dram_tensor("woutT", [128, IT, D], mdt, kind="ExternalInput").ap()
    bnet_d = nc.dram_tensor("bnet", [1, L, D], hdt, kind="ExternalInput").ap()
    onesc_d = nc.dram_tensor("onesc", [128, 128], mdt, kind="ExternalInput").ap()
    onesb_d = nc.dram_tensor("onesb", [1, 128], hdt, kind="ExternalInput").ap()
    out_d = nc.dram_tensor("out", [L, T, D], f32, kind="ExternalOutput").ap()

    def asf32(ap):
        # DVE/ACT read float32r tiles as plain fp32 bits
        return ap.bitcast(f32) if ap.dtype == f32r else ap

    with tile.TileContext(nc) as tc, ExitStack() as ctx:
        pc = ctx.enter_context(tc.tile_pool(name="const", bufs=1))
        pa = ctx.enter_context(tc.tile_pool(name="attp", bufs=1))
        pmm = ctx.enter_context(tc.tile_pool(name="psmm", bufs=3, space="PSUM"))
        ptp = ctx.enter_context(tc.tile_pool(name="pstp", bufs=2, space="PSUM"))
        psm = ctx.enter_context(tc.tile_pool(name="pssm", bufs=2, space="PSUM"))

        # ---- constants ----
        ident = pc.tile([128, 128], f32, tag="ident")
        make_identity(nc, ident)
        ident_b = pc.tile([128, 128], hdt, tag="ident_b")
        make_identity(nc, ident_b)
        ones_m = pc.tile([128, 128], mdt, tag="ones_m")          # f32r ones
        nc.sync.dma_start(ones_m[:], onesc_d[:])
        ones_b = pc.tile([1, 128], hdt, tag="ones_b")            # bf16 ones row
        nc.sync.dma_start(ones_b[:], onesb_d[:])
        ones_f = pc.tile([1, 2], f32, tag="ones_f")              # fp32 ones
        nc.vector.memset(ones_f[:], 1.0)
        eps_c = pc.tile([128, 1], f32, tag="eps")
        nc.vector.memset(eps_c[:], EPS)

        # whole-kernel attention state
        k_all = pa.tile([128, M, INNER], adt, tag="k_all")   # RAW k
        v_all = pa.tile([128, M, INNER], adt, tag="v_all")
        g_all = pa.tile([128, L, H], f32, tag="g_all")
        kss_all = pa.tile([128, M, H], f32, tag="kss_all")
        sim_all = pa.tile([128, L, H, M], f32, tag="sim_all")
        krms = pa.tile([128, M, H], f32, tag="krms")
        krinv = pa.tile([128, M, H], f32, tag="krinv")

        with ExitStack() as ctx_b:
            pb = ctx_b.enter_context(tc.tile_pool(name="bigact", bufs=1))
            ps_ = ctx_b.enter_context(tc.tile_pool(name="scr", bufs=2))

            xT = pb.tile([128, DT, LT], mdt, tag="xT")
            nc.sync.dma_start(xT[:], xT_d[:])
            xTb = pb.tile([128, DT, LT], hdt, tag="xTb")
            nc.sync.dma_start(xTb[:], xTb_d[:])
            wkv = pb.tile([128, DT, 2 * INNER], hdt, tag="wkv")
            nc.sync.dma_start(wkv[:], wkv_d[:])
            q_all = pb.tile([128, L, INNER], adt, tag="q_all")

            with ExitStack() as ctx_w:
                pw = ctx_w.enter_context(tc.tile_pool(name="qgp", bufs=1))
                wq = pw.tile([128, DT, INNER], mdt, tag="wq")
                nc.sync.dma_start(wq[:], wq_d[:])
                wg = pw.tile([128, DT, H], mdt, tag="wg")
                nc.sync.dma_start(wg[:], wg_d[:])

                # ---- phase A: token rms stats ----
                with nc.named_scope("stats"):
                    rms_row = pw.tile([1, LT], f32, tag="rms_row")
                    for c in range(2):
                        ssps = psm.tile([2, 512], f32, tag="sm")
                        for d in range(DT):
                            sq = ps_.tile([128, 512], mdt, tag="sq")
                            nc.vector.tensor_tensor(
                                sq[:], asf32(xT[:, d, c * 512:(c + 1) * 512]),
                                asf32(xT[:, d, c * 512:(c + 1) * 512]), AL.mult)
                            nc.tensor.matmul(ssps[:], ones_m[:, 0:2], sq[:],
                                             start=(d == 0), stop=(d == DT - 1))
                        nc.scalar.activation(rms_row[0:1, c * 512:(c + 1) * 512],
                                             ssps[0:1, :], AF.Sqrt,
                                             scale=1.0 / D, bias=eps_c[0:1, 0:1])
                    rs_row = pw.tile([1, LT], f32, tag="rs_row")
                    nc.vector.reciprocal(rs_row[:], rms_row[:])
                    # move scale to token-major: rs_tok[t, l]
                    rsps = psm.tile([128, L], f32, tag="sm")
                    for l in range(L):
                        nc.tensor.matmul(rsps[:, l:l + 1],
                                         rs_row[0:1, l * T:(l + 1) * T],
                                         ones_f[0:1, 0:1], start=True, stop=True)
                    rs_tok = pw.tile([128, L], f32, tag="rs_tok")
                    nc.vector.tensor_copy(rs_tok[:], rsps[:])

                # ---- phase D: queries and gates ----
                with nc.named_scope("qg"):
                    for l in range(L):
                        qps = pmm.tile([128, INNER], f32, tag="mm")
                        for d in range(DT):
                            nc.tensor.matmul(qps[:], xT[:, d, l * T:(l + 1) * T],
                                             wq[:, d], start=(d == 0),
                                             stop=(d == DT - 1))
                        nc.scalar.activation(q_all[:, l], qps[:], AF.Copy,
                                             scale=rs_tok[:, l:l + 1])
                    for l in range(L):
                        gps = psm.tile([128, H], f32, tag="sm")
                        for d in range(DT):
                            nc.tensor.matmul(gps[:], xT[:, d, l * T:(l + 1) * T],
                                             wg[:, d], start=(d == 0),
                                             stop=(d == DT - 1))
                        nc.scalar.activation(g_all[:, l], gps[:], AF.Sigmoid,
                                             scale=rs_tok[:, l:l + 1])

            # ---- phase C: member Linear + kv + sim products (bf16 stage) ----
            def kv_msg(lhsT_of_dt, m):
                for half in range(2):  # 0 = k, 1 = v
                    ps = pmm.tile([128, INNER], f32, tag="mm")
                    for d in range(DT):
                        nc.tensor.matmul(ps[:], lhsT_of_dt(d),
                                         wkv[:, d, half * INNER:(half + 1) * INNER],
                                         start=(d == 0), stop=(d == DT - 1))
                    if half == 0:
                        nc.scalar.copy(k_all[:, m], ps[:])
                        ksq = ps_.tile([128, INNER], f32, tag="ksq")
                        nc.gpsimd.tensor_tensor(ksq[:], asf32(k_all[:, m]),
                                                asf32(k_all[:, m]), AL.mult)
                        nc.vector.tensor_reduce(
                            kss_all[:, m], ksq.rearrange("p (h d) -> p h d", d=DH),
                            axis=AX.X, op=AL.add)
                    else:
                        nc.scalar.copy(v_all[:, m], ps[:])

            def sim_msg(m):
                for lq in range(L):
                    scr = ps_.tile([128, INNER], adt, tag="scr")
                    nc.vector.tensor_tensor(scr[:], asf32(q_all[:, lq]),
                                            asf32(k_all[:, m]), AL.mult)
                    nc.vector.tensor_reduce(
                        sim_all[:, lq, :, m],
                        scr.rearrange("p (h d) -> p h d", d=DH),
                        axis=AX.X, op=AL.add)

            with ExitStack() as ctx_l, nc.named_scope("linkv"):
                plk = ctx_l.enter_context(tc.tile_pool(name="linp", bufs=1))
                plw = ctx_l.enter_context(tc.tile_pool(name="linw", bufs=2))
                for l in range(L):
                    kv_msg(lambda d: xTb[:, d, l * T:(l + 1) * T], l)
                    sim_msg(l)
                    # member Linear, token-major out, in 2 chunks of 512 cols
                    bnet_l = plk.tile([1, D], hdt, tag="bnet_l")
                    nc.sync.dma_start(bnet_l[:], bnet_d[0:1, l])
                    osb = plk.tile([128, D], hdt, tag="osb")
                    for oc in range(2):
                        wnt = plw.tile([128, DT, 512], hdt, tag="wnt")
                        nc.sync.dma_start(wnt[:],
                                          wnet_d[l][:, :, oc * 512:(oc + 1) * 512])
                        ps = pmm.tile([128, 512], f32, tag="mm")
                        for d in range(DT):
                            nc.tensor.matmul(ps[:], xTb[:, d, l * T:(l + 1) * T],
                                             wnt[:, d], start=(d == 0), stop=False)
                        nc.tensor.matmul(ps[:], ones_b[0:1, :],
                                         bnet_l[0:1, oc * 512:(oc + 1) * 512],
                                         start=False, stop=True)
                        nc.scalar.copy(osb[:, oc * 512:(oc + 1) * 512], ps[:])
                    # transpose Linear output to feature-major (bf16)
                    oT = plk.tile([128, DT, T], hdt, tag="oT")
                    for ot in range(DT):
                        tps = ptp.tile([128, 128], hdt, tag="tp")
                        nc.tensor.transpose(tps[:], osb[:, ot * 128:(ot + 1) * 128],
                                            ident_b[:])
                        nc.scalar.copy(oT[:, ot], tps[:])
                    kv_msg(lambda d: oT[:, d], L + l)
                    sim_msg(L + l)

            # k rms scales (applied to sim, not k: dh-independent)
            nc.scalar.activation(krms.rearrange("p m h -> p (m h)"),
                                 kss_all.rearrange("p m h -> p (m h)"),
                                 AF.Sqrt, scale=1.0 / DH, bias=eps_c[:, 0:1])
            nc.vector.reciprocal(krinv.rearrange("p m h -> p (m h)"),
                                 krms.rearrange("p m h -> p (m h)"))

        # ---- phase E/F: batched softmax, then o + pooled per member ----
        with ExitStack() as ctx_e:
            pe = ctx_e.enter_context(tc.tile_pool(name="outp", bufs=1))
            pes = ctx_e.enter_context(tc.tile_pool(name="outs", bufs=2))
            wout = pe.tile([128, IT, D], mdt, tag="wout")
            nc.sync.dma_start(wout[:], wout_d[:])
            o_all = pe.tile([128, L, INNER], f32, tag="o_all")
            H2 = H // 2
            v_re = v_all.rearrange("p m (h d) -> p h d m", d=DH)
            with nc.named_scope("attn"):
                # batched softmax over all members at once
                nc.vector.tensor_tensor(
                    sim_all[:], sim_all[:],
                    krinv.rearrange("p m h -> p h m")[:, None]
                    .to_broadcast([128, L, H, M]), AL.mult)
                mx_all = pe.tile([128, L, H], f32, tag="mx_all")
                nc.vector.tensor_reduce(mx_all[:], sim_all[:], axis=AX.X, op=AL.max)
                nc.vector.tensor_tensor(
                    sim_all[:], sim_all[:],
                    mx_all[:, :, :, None].to_broadcast([128, L, H, M]), AL.subtract)
                pl_all = pe.tile([128, L, H, M], adt, tag="pl_all")
                nc.scalar.activation(pl_all.rearrange("p l h m -> p (l h m)"),
                                     sim_all.rearrange("p l h m -> p (l h m)"),
                                     AF.Exp)
                sm_all = pe.tile([128, L, H], f32, tag="sm_all")
                nc.vector.tensor_reduce(sm_all[:], asf32(pl_all[:]),
                                        axis=AX.X, op=AL.add)
                rgf = pe.tile([128, L, H], f32, tag="rgf")
                nc.vector.reciprocal(rgf.rearrange("p l h -> p (l h)"),
                                     sm_all.rearrange("p l h -> p (l h)"))
                rg = pe.tile([128, L, H], adt, tag="rg")
                nc.vector.tensor_tensor(rg[:], rgf[:], g_all[:], AL.mult)
                nc.vector.tensor_tensor(
                    pl_all[:], pl_all[:],
                    rg[:, :, :, None].to_broadcast([128, L, H, M]), AL.mult)
                for l in range(L):
                    o_l = o_all[:, l].rearrange("p (h d) -> p h d", d=DH)
                    for hh in range(2):
                        hs = slice(hh * H2, (hh + 1) * H2)
                        prod = pe.tile([128, H2, DH, M], adt, tag="prod")
                        nc.vector.tensor_tensor(
                            prod[:], v_re[:, hs],
                            pl_all[:, l, hs, None, :]
                            .to_broadcast([128, H2, DH, M]), AL.mult)
                        nc.vector.tensor_reduce(
                            o_l[:, hs], prod[:], axis=AX.X, op=AL.add)
                    # pooled = o @ wout
                    oTt = pes.tile([128, IT, T], mdt, tag="oTt")
                    for it in range(IT):
                        tps = ptp.tile([128, 128], f32, tag="tp")
                        nc.tensor.transpose(
                            tps[:], o_all[:, l, it * 128:(it + 1) * 128], ident[:])
                        nc.scalar.copy(oTt[:, it], tps[:])
                    pout = pes.tile([128, D], f32, tag="pout")
                    for oc in range(2):
                        ps = pmm.tile([128, 512], f32, tag="mm")
                        for it in range(IT):
                            nc.tensor.matmul(ps[:], oTt[:, it],
                                             wout[:, it, oc * 512:(oc + 1) * 512],
                                             start=(it == 0), stop=(it == IT - 1))
                        nc.scalar.copy(pout[:, oc * 512:(oc + 1) * 512], ps[:])
                    nc.sync.dma_start(out_d[l][:], pout[:])

    nc.compile()
    return nc


def get_nc():
    key = (MM_DT, ATT_DT)
    if key not in _NC_CACHE:
        _NC_CACHE[key] = _build(*key)
    return _NC_CACHE[key]


def prep_weights(w_net, b_net, norm_w, wq, wkv, knorm_w, wg, wout):
    """CPU-side layout prep shared by all cores (fp32 contiguous arrays)."""
    wnetT = np.ascontiguousarray(
        w_net.reshape(L, D, DT, 128).transpose(0, 3, 2, 1))          # [L,128,DT,D]
    colscale = (np.tile(knorm_w, H) * SCALE).astype(np.float32)
    wq2 = norm_w[:, None] * wq * colscale[None, :]
    wqT = np.ascontiguousarray(wq2.reshape(DT, 128, INNER).transpose(1, 0, 2))
    wkvT = np.ascontiguousarray(wkv.reshape(DT, 128, 2 * INNER).transpose(1, 0, 2))
    wgT = np.ascontiguousarray((norm_w[:, None] * wg).reshape(DT, 128, H)
                               .transpose(1, 0, 2))
    woutT = np.ascontiguousarray(wout.reshape(IT, 128, D).transpose(1, 0, 2))
    import ml_dtypes
    bf = ml_dtypes.bfloat16
    return dict(
        wnetT=wnetT.astype(bf),
        wqT=wqT.astype(np.float32),
        wkvT=wkvT.astype(bf),
        wgT=wgT.astype(np.float32),
        woutT=woutT.astype(np.float32),
        bnet=np.ascontiguousarray(b_net[None]).astype(bf),
        onesc=np.ones((128, 128), dtype=np.float32),
        onesb=np.ones((1, 128), dtype=bf),
    )


def prep_core_x(tokens, c):
    """Per-core feature-major token slice: [128, DT, LT]."""
    xs = tokens[:, :, c * NSL:(c + 1) * NSL, :].reshape(L, T, D)
    xT = xs.reshape(L, T, DT, 128).transpose(3, 2, 0, 1).reshape(128, DT, LT)
    return np.ascontiguousarray(xT).astype(np.float32)


def make_in_maps(tokens, w_net, b_net, norm_w, wq, wkv, knorm_w, wg, wout):
    shared = prep_weights(np.asarray(w_net, np.float32), np.asarray(b_net, np.float32),
                          np.asarray(norm_w, np.float32), np.asarray(wq, np.float32),
                          np.asarray(wkv, np.float32), np.asarray(knorm_w, np.float32),
                          np.asarray(wg, np.float32), np.asarray(wout, np.float32))
    import ml_dtypes
    tokens = np.asarray(tokens, np.float32)
    maps = []
    for c in range(NCORES):
        xT = prep_core_x(tokens, c)
        maps.append(dict(shared, xT=xT, xTb=xT.astype(ml_dtypes.bfloat16)))
    return maps


def stitch(results):
    full = np.empty((L, B, N, D), dtype=np.float32)
    for c in range(NCORES):
        full[:, :, c * NSL:(c + 1) * NSL, :] = \
            results[c]["out"].reshape(L, B, NSL, D)
    return full


def kernel(tokens, w_net, b_net, norm_w, wq, wkv, knorm_w, wg, wout):
    nc = get_nc()
    in_maps = make_in_maps(tokens, w_net, b_net, norm_w, wq, wkv, knorm_w, wg, wout)
    res = bass_utils.run_bass_kernel_spmd(nc, in_maps, core_ids=list(range(NCORES)))
    return stitch(res.results)


# revision 12
# speedup vs baseline: 1.5918x; 1.2796x over previous
"""Trainium2 Bass kernel for nn_EnsemblesWithMessagePassing.

Strategy: data-parallel over token positions (shard N=512 across the 8
NeuronCores, 64 positions each => 128 (b,n) tokens per core). The voting
attention is strictly per-position over the M=16 local messages, so this
sharding needs no collectives: each core holds every ensemble member's
weights and computes all 8 members for its position slice.

On-chip dataflow per core (feature-major activations for the PE):
  A) rms stats of tokens via DVE square + ones-column matmul (cross-
     partition sum), rsqrt; a [1,T]-lhsT rank-1 matmul moves the scale to
     token-major [T,1] so it fuses into the q/gates PSUM->SBUF copy.
  B) q = (x @ wq') * rs  and  gates = sigmoid((x @ wg') * rs); norm_w,
     knorm_w and the attention scale are pre-folded into wq'/wg' on CPU.
  C) per member l: kv for the token message; member Linear (token-major,
     bias added via a K=1 rank-1 matmul into the same PSUM accumulation
     group); PE-transpose of the Linear output; kv for the output message.
     k stays RAW -- its rms scale (dh-independent) is folded into sim
     later, so the sim products for a message are emitted immediately
     after its kv and overlap the remaining members' PE work.
  E) per member l: sim *= krinv, softmax over m (max-subtracted), gate
     folded into attn, o = reduce over m of attn*v, then PE-transpose of
     o and pooled = o @ wout, DMA straight from PSUM out.

Matmuls run as float32r (TF32-style datapath, fp32 PSUM accumulation);
everything else is fp32.
"""
import sys

for _p in ("/opt/trn_rl_repo", "/root/.axon_site/_ro/trn_rl_repo"):
    if _p not in sys.path:
        sys.path.insert(0, _p)

try:  # NTFF profile hook glue (only needed if BASS_TRACE is set externally)
    import antenv.axon_hooks  # noqa: F401
except Exception:
    try:
        import importlib.util as _ilu
        import antenv as _antenv

        _spec = _ilu.spec_from_file_location(
            "antenv.axon_hooks", "/opt/trn_rl_repo/antenv/axon_hooks.py")
        _mod = _ilu.module_from_spec(_spec)
        _spec.loader.exec_module(_mod)
        _antenv.axon_hooks = _mod
        sys.modules["antenv.axon_hooks"] = _mod
    except Exception:
        pass

from contextlib import ExitStack

import numpy as np

import concourse.bass as bass
import concourse.tile as tile
from concourse import bacc, mybir
from concourse import bass_utils
from concourse.masks import make_identity

f32 = mybir.dt.float32
f32r = mybir.dt.float32r
bf16 = mybir.dt.bfloat16
AF = mybir.ActivationFunctionType
AL = mybir.AluOpType
AX = mybir.AxisListType

# problem shape
L, B, N, D = 8, 2, 512, 1024
H, DH = 8, 64
INNER = H * DH          # 512
M = 2 * L               # 16 messages
SCALE = DH ** -0.5
EPS = float(np.finfo(np.float32).eps)

NCORES = 8
NSL = N // NCORES       # 64 positions per core per batch row
T = B * NSL             # 128 tokens per core
LT = L * T              # 1024
DT = D // 128           # 8 d-tiles
IT = INNER // 128       # 4 inner-tiles

# compute dtype config
MM_DT = "float32r"      # matmul operand dtype: "float32r" | "float32"
ATT_DT = "bfloat16"     # attention DVE dtype: "float32" | "bfloat16"

_NC_CACHE = {}


def _build(mm_dt_name=MM_DT, att_dt_name=ATT_DT):
    mdt = getattr(mybir.dt, mm_dt_name)   # light stages: stats, q, gates, pooled
    adt = getattr(mybir.dt, att_dt_name)  # attention elementwise dtype
    hdt = bf16                            # heavy stages: member Linear + kv

    nc = bacc.Bacc("TRN2", target_bir_lowering=False, debug=False,
                   enable_asserts=False, num_devices=NCORES)

    xT_d = nc.dram_tensor("xT", [128, DT, LT], mdt, kind="ExternalInput").ap()
    xTb_d = nc.dram_tensor("xTb", [128, DT, LT], hdt, kind="ExternalInput").ap()
    wnet_d = nc.dram_tensor("wnetT", [L, 128, DT, D], hdt, kind="ExternalInput").ap()
    wq_d = nc.dram_tensor("wqT", [128, DT, INNER], mdt, kind="ExternalInput").ap()
    wkv_d = nc.dram_tensor("wkvT", [128, DT, 2 * INNER], hdt, kind="ExternalInput").ap()
    wg_d = nc.dram_tensor("wgT", [128, DT, H], mdt, kind="ExternalInput").ap()
    wout_d = nc.dram_tensor("woutT", [128, IT, D], mdt, kind="ExternalInput").ap()
    bnet_d = nc.dram_tensor("bnet", [1, L, D], hdt, kind="ExternalInput").ap()
    onesc_d = nc.dram_tensor("onesc", [128, 128], mdt, kind="ExternalInput").ap()
    onesb_d = nc.dram_tensor("onesb", [1, 128], hdt, kind="ExternalInput").ap()
    out_d = nc.dram_tensor("out", [L, T, D], f32, kind="ExternalOutput").ap()

    def asf32(ap):
        # DVE/ACT read float32r tiles as plain fp32 bits
        return ap.bitcast(f32) if ap.dtype == f32r else ap

    with tile.TileContext(nc) as tc, ExitStack() as ctx:
        pc = ctx.enter_context(tc.tile_pool(name="const", bufs=1))
        pa = ctx.enter_context(tc.tile_pool(name="attp", bufs=1))
        pmm = ctx.enter_context(tc.tile_pool(name="psmm", bufs=4, space="PSUM"))
        ptp = ctx.enter_context(tc.tile_pool(name="pstp", bufs=2, space="PSUM"))
        psm = ctx.enter_context(tc.tile_pool(name="pssm", bufs=2, space="PSUM"))

        # ---- constants ----
        ident = pc.tile([128, 128], f32, tag="ident")
        make_identity(nc, ident)
        ident_b = pc.tile([128, 128], hdt, tag="ident_b")
        make_identity(nc, ident_b)
        ones_m = pc.tile([128, 128], mdt, tag="ones_m")          # f32r ones
        nc.sync.dma_start(ones_m[:], onesc_d[:])
        ones_b = pc.tile([1, 128], hdt, tag="ones_b")            # bf16 ones row
        nc.sync.dma_start(ones_b[:], onesb_d[:])
        ones_f = pc.tile([1, 2], f32, tag="ones_f")              # fp32 ones
        nc.vector.memset(ones_f[:], 1.0)
        eps_c = pc.tile([128, 1], f32, tag="eps")
        nc.vector.memset(eps_c[:], EPS)

        # whole-kernel attention state
        k_all = pa.tile([128, M, INNER], adt, tag="k_all")   # RAW k
        v_allT = pa.tile([128, H, DH, M], adt, tag="v_allT")
        g_all = pa.tile([128, L, H], f32, tag="g_all")
        kss_all = pa.tile([128, M, H], f32, tag="kss_all")
        sim_all = pa.tile([128, L, H, M], f32, tag="sim_all")
        krms = pa.tile([128, M, H], f32, tag="krms")
        krinv = pa.tile([128, M, H], f32, tag="krinv")

        with ExitStack() as ctx_b:
            pb = ctx_b.enter_context(tc.tile_pool(name="bigact", bufs=1))
            ps_ = ctx_b.enter_context(tc.tile_pool(name="scr", bufs=2))

            xT = pb.tile([128, DT, LT], mdt, tag="xT")
            nc.sync.dma_start(xT[:], xT_d[:])
            xTb = pb.tile([128, DT, LT], hdt, tag="xTb")
            nc.sync.dma_start(xTb[:], xTb_d[:])
            wkv = pb.tile([128, DT, 2 * INNER], hdt, tag="wkv")
            nc.sync.dma_start(wkv[:], wkv_d[:])
            q_all = pb.tile([128, L, INNER], adt, tag="q_all")

            with ExitStack() as ctx_w:
                pw = ctx_w.enter_context(tc.tile_pool(name="qgp", bufs=1))
                wq = pw.tile([128, DT, INNER], mdt, tag="wq")
                nc.sync.dma_start(wq[:], wq_d[:])
                wg = pw.tile([128, DT, H], mdt, tag="wg")
                nc.sync.dma_start(wg[:], wg_d[:])

                # ---- phase A: token rms stats ----
                with nc.named_scope("stats"):
                    rms_row = pw.tile([1, LT], f32, tag="rms_row")
                    for c in range(2):
                        ssps = psm.tile([2, 512], f32, tag="sm")
                        for d in range(DT):
                            sq = ps_.tile([128, 512], mdt, tag="sq")
                            nc.vector.tensor_tensor(
                                sq[:], asf32(xT[:, d, c * 512:(c + 1) * 512]),
                                asf32(xT[:, d, c * 512:(c + 1) * 512]), AL.mult)
                            nc.tensor.matmul(ssps[:], ones_m[:, 0:2], sq[:],
                                             start=(d == 0), stop=(d == DT - 1))
                        nc.scalar.activation(rms_row[0:1, c * 512:(c + 1) * 512],
                                             ssps[0:1, :], AF.Sqrt,
                                             scale=1.0 / D, bias=eps_c[0:1, 0:1])
                    rs_row = pw.tile([1, LT], f32, tag="rs_row")
                    nc.vector.reciprocal(rs_row[:], rms_row[:])
                    # move scale to token-major: rs_tok[t, l]
                    rsps = psm.tile([128, L], f32, tag="sm")
                    for l in range(L):
                        nc.tensor.matmul(rsps[:, l:l + 1],
                                         rs_row[0:1, l * T:(l + 1) * T],
                                         ones_f[0:1, 0:1], start=True, stop=True)
                    rs_tok = pw.tile([128, L], f32, tag="rs_tok")
                    nc.vector.tensor_copy(rs_tok[:], rsps[:])

                # ---- phase D: queries and gates ----
                with nc.named_scope("qg"):
                    for l in range(L):
                        qps = pmm.tile([128, INNER], f32, tag="mm")
                        for d in range(DT):
                            nc.tensor.matmul(qps[:], xT[:, d, l * T:(l + 1) * T],
                                             wq[:, d], start=(d == 0),
                                             stop=(d == DT - 1))
                        nc.scalar.activation(q_all[:, l], qps[:], AF.Copy,
                                             scale=rs_tok[:, l:l + 1])
                    for l in range(L):
                        gps = psm.tile([128, H], f32, tag="sm")
                        for d in range(DT):
                            nc.tensor.matmul(gps[:], xT[:, d, l * T:(l + 1) * T],
                                             wg[:, d], start=(d == 0),
                                             stop=(d == DT - 1))
                        nc.scalar.activation(g_all[:, l], gps[:], AF.Sigmoid,
                                             scale=rs_tok[:, l:l + 1])

            # ---- phase C: member Linear + kv + sim products (bf16 stage) ----
            def kv_msg(lhsT_of_dt, m):
                for half in range(2):  # 0 = k, 1 = v
                    ps = pmm.tile([128, INNER], f32, tag="mm")
                    for d in range(DT):
                        nc.tensor.matmul(ps[:], lhsT_of_dt(d),
                                         wkv[:, d, half * INNER:(half + 1) * INNER],
                                         start=(d == 0), stop=(d == DT - 1))
                    if half == 0:
                        nc.scalar.copy(k_all[:, m], ps[:])
                        ksq = ps_.tile([128, INNER], f32, tag="ksq")
                        nc.gpsimd.tensor_tensor(ksq[:], asf32(k_all[:, m]),
                                                asf32(k_all[:, m]), AL.mult)
                        nc.vector.tensor_reduce(
                            kss_all[:, m], ksq.rearrange("p (h d) -> p h d", d=DH),
                            axis=AX.X, op=AL.add)
                    else:
                        nc.scalar.copy(v_allT[:, :, :, m],
                                       ps.rearrange("p (h d) -> p h d", d=DH))

            def sim_msg(m):
                for lq in range(L):
                    scr = ps_.tile([128, INNER], adt, tag="scr")
                    nc.vector.tensor_tensor(scr[:], asf32(q_all[:, lq]),
                                            asf32(k_all[:, m]), AL.mult)
                    nc.vector.tensor_reduce(
                        sim_all[:, lq, :, m],
                        scr.rearrange("p (h d) -> p h d", d=DH),
                        axis=AX.X, op=AL.add)

            with ExitStack() as ctx_l, nc.named_scope("linkv"):
                plk = ctx_l.enter_context(tc.tile_pool(name="linp", bufs=1))
                plw = ctx_l.enter_context(tc.tile_pool(name="linw", bufs=2))
                for l in range(L):
                    kv_msg(lambda d: xTb[:, d, l * T:(l + 1) * T], l)
                    sim_msg(l)
                    # member Linear, token-major out, in 2 chunks of 512 cols
                    bnet_l = plk.tile([1, D], hdt, tag="bnet_l")
                    nc.sync.dma_start(bnet_l[:], bnet_d[0:1, l])
                    osb = plk.tile([128, D], hdt, tag="osb")
                    for oc in range(2):
                        wnt = plw.tile([128, DT, 512], hdt, tag="wnt")
                        nc.sync.dma_start(wnt[:],
                                          wnet_d[l][:, :, oc * 512:(oc + 1) * 512])
                        ps = pmm.tile([128, 512], f32, tag="mm")
                        for d in range(DT):
                            nc.tensor.matmul(ps[:], xTb[:, d, l * T:(l + 1) * T],
                                             wnt[:, d], start=(d == 0), stop=False)
                        nc.tensor.matmul(ps[:], ones_b[0:1, :],
                                         bnet_l[0:1, oc * 512:(oc + 1) * 512],
                                         start=False, stop=True)
                        nc.scalar.copy(osb[:, oc * 512:(oc + 1) * 512], ps[:])
                    # transpose Linear output to feature-major (bf16)
                    oT = plk.tile([128, DT, T], hdt, tag="oT")
                    for ot in range(DT):
                        tps = ptp.tile([128, 128], hdt, tag="tp")
                        nc.tensor.transpose(tps[:], osb[:, ot * 128:(ot + 1) * 128],
                                            ident_b[:])
                        nc.scalar.copy(oT[:, ot], tps[:])
                    kv_msg(lambda d: oT[:, d], L + l)
                    sim_msg(L + l)

            # k rms scales (applied to sim, not k: dh-independent)
            nc.scalar.activation(krms.rearrange("p m h -> p (m h)"),
                                 kss_all.rearrange("p m h -> p (m h)"),
                                 AF.Sqrt, scale=1.0 / DH, bias=eps_c[:, 0:1])
            nc.vector.reciprocal(krinv.rearrange("p m h -> p (m h)"),
                                 krms.rearrange("p m h -> p (m h)"))

        # ---- phase E/F: batched softmax, then o + pooled per member ----
        with ExitStack() as ctx_e:
            pe = ctx_e.enter_context(tc.tile_pool(name="outp", bufs=1))
            pes = ctx_e.enter_context(tc.tile_pool(name="outs", bufs=2))
            wout = pe.tile([128, IT, D], mdt, tag="wout")
            nc.sync.dma_start(wout[:], wout_d[:])
            o_all = pe.tile([128, L, INNER], f32, tag="o_all")
            H2 = H // 2
            with nc.named_scope("attn"):
                # batched softmax over all members at once
                nc.vector.tensor_tensor(
                    sim_all[:], sim_all[:],
                    krinv.rearrange("p m h -> p h m")[:, None]
                    .to_broadcast([128, L, H, M]), AL.mult)
                mx_all = pe.tile([128, L, H], f32, tag="mx_all")
                nc.vector.tensor_reduce(mx_all[:], sim_all[:], axis=AX.X, op=AL.max)
                nc.vector.tensor_tensor(
                    sim_all[:], sim_all[:],
                    mx_all[:, :, :, None].to_broadcast([128, L, H, M]), AL.subtract)
                pl_all = pe.tile([128, L, H, M], adt, tag="pl_all")
                nc.scalar.activation(pl_all.rearrange("p l h m -> p (l h m)"),
                                     sim_all.rearrange("p l h m -> p (l h m)"),
                                     AF.Exp)
                sm_all = pe.tile([128, L, H], f32, tag="sm_all")
                nc.vector.tensor_reduce(sm_all[:], asf32(pl_all[:]),
                                        axis=AX.X, op=AL.add)
                rgf = pe.tile([128, L, H], f32, tag="rgf")
                nc.vector.reciprocal(rgf.rearrange("p l h -> p (l h)"),
                                     sm_all.rearrange("p l h -> p (l h)"))
                rg = pe.tile([128, L, H], adt, tag="rg")
                nc.vector.tensor_tensor(rg[:], rgf[:], g_all[:], AL.mult)
                nc.vector.tensor_tensor(
                    pl_all[:], pl_all[:],
                    rg[:, :, :, None].to_broadcast([128, L, H, M]), AL.mult)
                for l in range(L):
                    o_l = o_all[:, l].rearrange("p (h d) -> p h d", d=DH)
                    for hh in range(2):
                        hs = slice(hh * H2, (hh + 1) * H2)
                        prod = pe.tile([128, H2, DH, M], adt, tag="prod")
                        nc.vector.tensor_tensor(
                            prod[:], v_allT[:, hs],
                            pl_all[:, l, hs, None, :]
                            .to_broadcast([128, H2, DH, M]), AL.mult)
                        half = M // 2
                        nc.vector.tensor_tensor(
                            prod[:, :, :, 0:half], prod[:, :, :, 0:half],
                            prod[:, :, :, half:M], AL.add)
                        nc.vector.tensor_reduce(
                            o_l[:, hs], prod[:, :, :, 0:half],
                            axis=AX.X, op=AL.add)
                    # pooled = o @ wout
                    oTt = pes.tile([128, IT, T], mdt, tag="oTt")
                    for it in range(IT):
                        tps = ptp.tile([128, 128], f32, tag="tp")
                        nc.tensor.transpose(
                            tps[:], o_all[:, l, it * 128:(it + 1) * 128], ident[:])
                        nc.scalar.copy(oTt[:, it], tps[:])
                    pout = pes.tile([128, D], f32, tag="pout")
                    for oc in range(2):
                        ps = pmm.tile([128, 512], f32, tag="mm")
                        for it in range(IT):
                            nc.tensor.matmul(ps[:], oTt[:, it],
                                             wout[:, it, oc * 512:(oc + 1) * 512],
                                             start=(it == 0), stop=(it == IT - 1))
                        nc.scalar.copy(pout[:, oc * 512:(oc + 1) * 512], ps[:])
                    nc.sync.dma_start(out_d[l][:], pout[:])

    nc.compile()
    return nc


def get_nc():
    key = (MM_DT, ATT_DT)
    if key not in _NC_CACHE:
        _NC_CACHE[key] = _build(*key)
    return _NC_CACHE[key]


def prep_weights(w_net, b_net, norm_w, wq, wkv, knorm_w, wg, wout):
    """CPU-side layout prep shared by all cores (fp32 contiguous arrays)."""
    wnetT = np.ascontiguousarray(
        w_net.reshape(L, D, DT, 128).transpose(0, 3, 2, 1))          # [L,128,DT,D]
    colscale = (np.tile(knorm_w, H) * SCALE).astype(np.float32)
    wq2 = norm_w[:, None] * wq * colscale[None, :]
    wqT = np.ascontiguousarray(wq2.reshape(DT, 128, INNER).transpose(1, 0, 2))
    wkvT = np.ascontiguousarray(wkv.reshape(DT, 128, 2 * INNER).transpose(1, 0, 2))
    wgT = np.ascontiguousarray((norm_w[:, None] * wg).reshape(DT, 128, H)
                               .transpose(1, 0, 2))
    woutT = np.ascontiguousarray(wout.reshape(IT, 128, D).transpose(1, 0, 2))
    import ml_dtypes
    bf = ml_dtypes.bfloat16
    return dict(
        wnetT=wnetT.astype(bf),
        wqT=wqT.astype(np.float32),
        wkvT=wkvT.astype(bf),
        wgT=wgT.astype(np.float32),
        woutT=woutT.astype(np.float32),
        bnet=np.ascontiguousarray(b_net[None]).astype(bf),
        onesc=np.ones((128, 128), dtype=np.float32),
        onesb=np.ones((1, 128), dtype=bf),
    )


def prep_core_x(tokens, c):
    """Per-core feature-major token slice: [128, DT, LT]."""
    xs = tokens[:, :, c * NSL:(c + 1) * NSL, :].reshape(L, T, D)
    xT = xs.reshape(L, T, DT, 128).transpose(3, 2, 0, 1).reshape(128, DT, LT)
    return np.ascontiguousarray(xT).astype(np.float32)


def make_in_maps(tokens, w_net, b_net, norm_w, wq, wkv, knorm_w, wg, wout):
    shared = prep_weights(np.asarray(w_net, np.float32), np.asarray(b_net, np.float32),
                          np.asarray(norm_w, np.float32), np.asarray(wq, np.float32),
                          np.asarray(wkv, np.float32), np.asarray(knorm_w, np.float32),
                          np.asarray(wg, np.float32), np.asarray(wout, np.float32))
    import ml_dtypes
    tokens = np.asarray(tokens, np.float32)
    maps = []
    for c in range(NCORES):
        xT = prep_core_x(tokens, c)
        maps.append(dict(shared, xT=xT, xTb=xT.astype(ml_dtypes.bfloat16)))
    return maps


def stitch(results):
    full = np.empty((L, B, N, D), dtype=np.float32)
    for c in range(NCORES):
        full[:, :, c * NSL:(c + 1) * NSL, :] = \
            results[c]["out"].reshape(L, B, NSL, D)
    return full


def kernel(tokens, w_net, b_net, norm_w, wq, wkv, knorm_w, wg, wout):
    nc = get_nc()
    in_maps = make_in_maps(tokens, w_net, b_net, norm_w, wq, wkv, knorm_w, wg, wout)
    res = bass_utils.run_bass_kernel_spmd(nc, in_maps, core_ids=list(range(NCORES)))
    return stitch(res.results)


# revision 13
# speedup vs baseline: 1.6795x; 1.0551x over previous
"""Trainium2 Bass kernel for nn_EnsemblesWithMessagePassing.

Strategy: data-parallel over token positions (shard N=512 across the 8
NeuronCores, 64 positions each => 128 (b,n) tokens per core). The voting
attention is strictly per-position over the M=16 local messages, so this
sharding needs no collectives: each core holds every ensemble member's
weights and computes all 8 members for its position slice.

On-chip dataflow per core (feature-major activations for the PE):
  A) rms stats of tokens via DVE square + ones-column matmul (cross-
     partition sum), rsqrt; a [1,T]-lhsT rank-1 matmul moves the scale to
     token-major [T,1] so it fuses into the q/gates PSUM->SBUF copy.
  B) q = (x @ wq') * rs  and  gates = sigmoid((x @ wg') * rs); norm_w,
     knorm_w and the attention scale are pre-folded into wq'/wg' on CPU.
  C) per member l: kv for the token message; member Linear (token-major,
     bias added via a K=1 rank-1 matmul into the same PSUM accumulation
     group); PE-transpose of the Linear output; kv for the output message.
     k stays RAW -- its rms scale (dh-independent) is folded into sim
     later, so the sim products for a message are emitted immediately
     after its kv and overlap the remaining members' PE work.
  E) per member l: sim *= krinv, softmax over m (max-subtracted), gate
     folded into attn, o = reduce over m of attn*v, then PE-transpose of
     o and pooled = o @ wout, DMA straight from PSUM out.

Matmuls run as float32r (TF32-style datapath, fp32 PSUM accumulation);
everything else is fp32.
"""
import sys

for _p in ("/opt/trn_rl_repo", "/root/.axon_site/_ro/trn_rl_repo"):
    if _p not in sys.path:
        sys.path.insert(0, _p)

try:  # NTFF profile hook glue (only needed if BASS_TRACE is set externally)
    import antenv.axon_hooks  # noqa: F401
except Exception:
    try:
        import importlib.util as _ilu
        import antenv as _antenv

        _spec = _ilu.spec_from_file_location(
            "antenv.axon_hooks", "/opt/trn_rl_repo/antenv/axon_hooks.py")
        _mod = _ilu.module_from_spec(_spec)
        _spec.loader.exec_module(_mod)
        _antenv.axon_hooks = _mod
        sys.modules["antenv.axon_hooks"] = _mod
    except Exception:
        pass

from contextlib import ExitStack

import numpy as np

import concourse.bass as bass
import concourse.tile as tile
from concourse import bacc, mybir
from concourse import bass_utils
from concourse.masks import make_identity

f32 = mybir.dt.float32
f32r = mybir.dt.float32r
bf16 = mybir.dt.bfloat16
AF = mybir.ActivationFunctionType
AL = mybir.AluOpType
AX = mybir.AxisListType

# problem shape
L, B, N, D = 8, 2, 512, 1024
H, DH = 8, 64
INNER = H * DH          # 512
M = 2 * L               # 16 messages
SCALE = DH ** -0.5
EPS = float(np.finfo(np.float32).eps)

NCORES = 8
NSL = N // NCORES       # 64 positions per core per batch row
T = B * NSL             # 128 tokens per core
LT = L * T              # 1024
DT = D // 128           # 8 d-tiles
IT = INNER // 128       # 4 inner-tiles

# compute dtype config
MM_DT = "float32r"      # matmul operand dtype: "float32r" | "float32"
ATT_DT = "bfloat16"     # attention DVE dtype: "float32" | "bfloat16"

_NC_CACHE = {}


def _build(mm_dt_name=MM_DT, att_dt_name=ATT_DT):
    mdt = getattr(mybir.dt, mm_dt_name)   # light stages: stats, q, gates, pooled
    adt = getattr(mybir.dt, att_dt_name)  # attention elementwise dtype
    hdt = bf16                            # heavy stages: member Linear + kv

    nc = bacc.Bacc("TRN2", target_bir_lowering=False, debug=False,
                   enable_asserts=False, num_devices=NCORES)

    xT_d = nc.dram_tensor("xT", [128, DT, LT], mdt, kind="ExternalInput").ap()
    xTb_d = nc.dram_tensor("xTb", [128, DT, LT], hdt, kind="ExternalInput").ap()
    wnet_d = nc.dram_tensor("wnetT", [L, 128, DT, D], hdt, kind="ExternalInput").ap()
    wq_d = nc.dram_tensor("wqT", [128, DT, INNER], mdt, kind="ExternalInput").ap()
    wkv_d = nc.dram_tensor("wkvT", [128, DT, 2 * INNER], hdt, kind="ExternalInput").ap()
    wg_d = nc.dram_tensor("wgT", [128, DT, H], mdt, kind="ExternalInput").ap()
    wout_d = nc.dram_tensor("woutT", [128, IT, D], mdt, kind="ExternalInput").ap()
    bnet_d = nc.dram_tensor("bnet", [1, L, D], hdt, kind="ExternalInput").ap()
    onesc_d = nc.dram_tensor("onesc", [128, 128], mdt, kind="ExternalInput").ap()
    onesb_d = nc.dram_tensor("onesb", [1, 128], hdt, kind="ExternalInput").ap()
    out_d = nc.dram_tensor("out", [L, T, D], f32, kind="ExternalOutput").ap()

    def asf32(ap):
        # DVE/ACT read float32r tiles as plain fp32 bits
        return ap.bitcast(f32) if ap.dtype == f32r else ap

    with tile.TileContext(nc) as tc, ExitStack() as ctx:
        pc = ctx.enter_context(tc.tile_pool(name="const", bufs=1))
        pa = ctx.enter_context(tc.tile_pool(name="attp", bufs=1))
        pmm = ctx.enter_context(tc.tile_pool(name="psmm", bufs=4, space="PSUM"))
        ptp = ctx.enter_context(tc.tile_pool(name="pstp", bufs=2, space="PSUM"))
        psm = ctx.enter_context(tc.tile_pool(name="pssm", bufs=2, space="PSUM"))

        # ---- constants ----
        ident = pc.tile([128, 128], f32, tag="ident")
        make_identity(nc, ident)
        ident_b = pc.tile([128, 128], hdt, tag="ident_b")
        make_identity(nc, ident_b)
        ones_m = pc.tile([128, 128], mdt, tag="ones_m")          # f32r ones
        nc.sync.dma_start(ones_m[:], onesc_d[:])
        ones_b = pc.tile([1, 128], hdt, tag="ones_b")            # bf16 ones row
        nc.sync.dma_start(ones_b[:], onesb_d[:])
        ones_f = pc.tile([1, 2], f32, tag="ones_f")              # fp32 ones
        nc.vector.memset(ones_f[:], 1.0)
        eps_c = pc.tile([128, 1], f32, tag="eps")
        nc.vector.memset(eps_c[:], EPS)

        # whole-kernel attention state
        k_all = pa.tile([128, M, INNER], adt, tag="k_all")   # RAW k
        v_allT = pa.tile([128, H, DH, M], adt, tag="v_allT")
        g_all = pa.tile([128, L, H], f32, tag="g_all")
        kss_all = pa.tile([128, M, H], f32, tag="kss_all")
        sim_all = pa.tile([128, L, H, M], f32, tag="sim_all")
        krms = pa.tile([128, M, H], f32, tag="krms")
        krinv = pa.tile([128, M, H], f32, tag="krinv")

        with ExitStack() as ctx_b:
            pb = ctx_b.enter_context(tc.tile_pool(name="bigact", bufs=1))
            ps_ = ctx_b.enter_context(tc.tile_pool(name="scr", bufs=2))

            xT = pb.tile([128, DT, LT], mdt, tag="xT")
            nc.sync.dma_start(xT[:], xT_d[:])
            xTb = pb.tile([128, DT, LT], hdt, tag="xTb")
            nc.sync.dma_start(xTb[:], xTb_d[:])
            wkv = pb.tile([128, DT, 2 * INNER], hdt, tag="wkv")
            nc.sync.dma_start(wkv[:], wkv_d[:])
            q_all = pb.tile([128, L, INNER], adt, tag="q_all")

            with ExitStack() as ctx_w:
                pw = ctx_w.enter_context(tc.tile_pool(name="qgp", bufs=1))
                wq = pw.tile([128, DT, INNER], mdt, tag="wq")
                nc.sync.dma_start(wq[:], wq_d[:])
                wg = pw.tile([128, DT, H], mdt, tag="wg")
                nc.sync.dma_start(wg[:], wg_d[:])

                # ---- phase A: token rms stats ----
                with nc.named_scope("stats"):
                    rms_row = pw.tile([1, LT], f32, tag="rms_row")
                    for c in range(2):
                        ssps = psm.tile([2, 512], f32, tag="sm")
                        for d in range(DT):
                            sq = ps_.tile([128, 512], mdt, tag="sq")
                            nc.vector.tensor_tensor(
                                sq[:], asf32(xT[:, d, c * 512:(c + 1) * 512]),
                                asf32(xT[:, d, c * 512:(c + 1) * 512]), AL.mult)
                            nc.tensor.matmul(ssps[:], ones_m[:, 0:2], sq[:],
                                             start=(d == 0), stop=(d == DT - 1))
                        nc.scalar.activation(rms_row[0:1, c * 512:(c + 1) * 512],
                                             ssps[0:1, :], AF.Sqrt,
                                             scale=1.0 / D, bias=eps_c[0:1, 0:1])
                    rs_row = pw.tile([1, LT], f32, tag="rs_row")
                    nc.vector.reciprocal(rs_row[:], rms_row[:])
                    # move scale to token-major: rs_tok[t, l]
                    rsps = psm.tile([128, L], f32, tag="sm")
                    for l in range(L):
                        nc.tensor.matmul(rsps[:, l:l + 1],
                                         rs_row[0:1, l * T:(l + 1) * T],
                                         ones_f[0:1, 0:1], start=True, stop=True)
                    rs_tok = pw.tile([128, L], f32, tag="rs_tok")
                    nc.vector.tensor_copy(rs_tok[:], rsps[:])

                # ---- phase D: queries and gates ----
                with nc.named_scope("qg"):
                    for l in range(L):
                        qps = pmm.tile([128, INNER], f32, tag="mm")
                        for d in range(DT):
                            nc.tensor.matmul(qps[:], xT[:, d, l * T:(l + 1) * T],
                                             wq[:, d], start=(d == 0),
                                             stop=(d == DT - 1))
                        nc.scalar.activation(q_all[:, l], qps[:], AF.Copy,
                                             scale=rs_tok[:, l:l + 1])
                    for l in range(L):
                        gps = psm.tile([128, H], f32, tag="sm")
                        for d in range(DT):
                            nc.tensor.matmul(gps[:], xT[:, d, l * T:(l + 1) * T],
                                             wg[:, d], start=(d == 0),
                                             stop=(d == DT - 1))
                        nc.scalar.activation(g_all[:, l], gps[:], AF.Sigmoid,
                                             scale=rs_tok[:, l:l + 1])

            # ---- phase C: member Linear + kv + sim products (bf16 stage) ----
            def kv_msg(lhsT_of_dt, m):
                for half in range(2):  # 0 = k, 1 = v
                    ps = pmm.tile([128, INNER], f32, tag="mm")
                    for d in range(DT):
                        nc.tensor.matmul(ps[:], lhsT_of_dt(d),
                                         wkv[:, d, half * INNER:(half + 1) * INNER],
                                         start=(d == 0), stop=(d == DT - 1))
                    if half == 0:
                        nc.scalar.copy(k_all[:, m], ps[:])
                        ksq = ps_.tile([128, INNER], f32, tag="ksq")
                        nc.gpsimd.tensor_tensor(ksq[:], asf32(k_all[:, m]),
                                                asf32(k_all[:, m]), AL.mult)
                        nc.vector.tensor_reduce(
                            kss_all[:, m], ksq.rearrange("p (h d) -> p h d", d=DH),
                            axis=AX.X, op=AL.add)
                    else:
                        nc.scalar.copy(v_allT[:, :, :, m],
                                       ps.rearrange("p (h d) -> p h d", d=DH))

            def sim_msg(m):
                scr8 = ps_.tile([128, L, INNER], adt, tag="scr8")
                for lq in range(L):
                    nc.vector.tensor_tensor(scr8[:, lq], asf32(q_all[:, lq]),
                                            asf32(k_all[:, m]), AL.mult)
                sv = scr8.rearrange("p l (h d) -> p l h d", d=DH)
                nc.vector.tensor_tensor(sv[:, :, :, 0:DH // 2],
                                        sv[:, :, :, 0:DH // 2],
                                        sv[:, :, :, DH // 2:DH], AL.add)
                nc.vector.tensor_reduce(
                    sim_all[:, :, :, m], sv[:, :, :, 0:DH // 2],
                    axis=AX.X, op=AL.add)

            with ExitStack() as ctx_l, nc.named_scope("linkv"):
                plk = ctx_l.enter_context(tc.tile_pool(name="linp", bufs=1))
                plw = ctx_l.enter_context(tc.tile_pool(name="linw", bufs=2))
                for l in range(L):
                    kv_msg(lambda d: xTb[:, d, l * T:(l + 1) * T], l)
                    sim_msg(l)
                    # member Linear, token-major out, in 2 chunks of 512 cols
                    bnet_l = plk.tile([1, D], hdt, tag="bnet_l")
                    nc.sync.dma_start(bnet_l[:], bnet_d[0:1, l])
                    osb = plk.tile([128, D], hdt, tag="osb")
                    for oc in range(2):
                        wnt = plw.tile([128, DT, 512], hdt, tag="wnt")
                        nc.sync.dma_start(wnt[:],
                                          wnet_d[l][:, :, oc * 512:(oc + 1) * 512])
                        ps = pmm.tile([128, 512], f32, tag="mm")
                        for d in range(DT):
                            nc.tensor.matmul(ps[:], xTb[:, d, l * T:(l + 1) * T],
                                             wnt[:, d], start=(d == 0), stop=False)
                        nc.tensor.matmul(ps[:], ones_b[0:1, :],
                                         bnet_l[0:1, oc * 512:(oc + 1) * 512],
                                         start=False, stop=True)
                        nc.scalar.copy(osb[:, oc * 512:(oc + 1) * 512], ps[:])
                    # transpose Linear output to feature-major (bf16)
                    oT = plk.tile([128, DT, T], hdt, tag="oT")
                    for ot in range(DT):
                        tps = ptp.tile([128, 128], hdt, tag="tp")
                        nc.tensor.transpose(tps[:], osb[:, ot * 128:(ot + 1) * 128],
                                            ident_b[:])
                        nc.scalar.copy(oT[:, ot], tps[:])
                    kv_msg(lambda d: oT[:, d], L + l)
                    sim_msg(L + l)

            # k rms scales (applied to sim, not k: dh-independent)
            nc.scalar.activation(krms.rearrange("p m h -> p (m h)"),
                                 kss_all.rearrange("p m h -> p (m h)"),
                                 AF.Sqrt, scale=1.0 / DH, bias=eps_c[:, 0:1])
            nc.vector.reciprocal(krinv.rearrange("p m h -> p (m h)"),
                                 krms.rearrange("p m h -> p (m h)"))

        # ---- phase E/F: batched softmax, then o + pooled per member ----
        with ExitStack() as ctx_e:
            pe = ctx_e.enter_context(tc.tile_pool(name="outp", bufs=1))
            pes = ctx_e.enter_context(tc.tile_pool(name="outs", bufs=2))
            wout = pe.tile([128, IT, D], mdt, tag="wout")
            nc.sync.dma_start(wout[:], wout_d[:])
            o_all = pe.tile([128, L, INNER], f32, tag="o_all")
            H2 = H // 2
            with nc.named_scope("attn"):
                # batched softmax over all members at once
                nc.vector.tensor_tensor(
                    sim_all[:], sim_all[:],
                    krinv.rearrange("p m h -> p h m")[:, None]
                    .to_broadcast([128, L, H, M]), AL.mult)
                mx_all = pe.tile([128, L, H], f32, tag="mx_all")
                nc.vector.tensor_reduce(mx_all[:], sim_all[:], axis=AX.X, op=AL.max)
                nc.vector.tensor_tensor(
                    sim_all[:], sim_all[:],
                    mx_all[:, :, :, None].to_broadcast([128, L, H, M]), AL.subtract)
                pl_all = pe.tile([128, L, H, M], adt, tag="pl_all")
                nc.scalar.activation(pl_all.rearrange("p l h m -> p (l h m)"),
                                     sim_all.rearrange("p l h m -> p (l h m)"),
                                     AF.Exp)
                sm_all = pe.tile([128, L, H], f32, tag="sm_all")
                nc.vector.tensor_reduce(sm_all[:], asf32(pl_all[:]),
                                        axis=AX.X, op=AL.add)
                rgf = pe.tile([128, L, H], f32, tag="rgf")
                nc.vector.reciprocal(rgf.rearrange("p l h -> p (l h)"),
                                     sm_all.rearrange("p l h -> p (l h)"))
                rg = pe.tile([128, L, H], adt, tag="rg")
                nc.vector.tensor_tensor(rg[:], rgf[:], g_all[:], AL.mult)
                nc.vector.tensor_tensor(
                    pl_all[:], pl_all[:],
                    rg[:, :, :, None].to_broadcast([128, L, H, M]), AL.mult)
                for l in range(L):
                    o_l = o_all[:, l].rearrange("p (h d) -> p h d", d=DH)
                    for hh in range(2):
                        hs = slice(hh * H2, (hh + 1) * H2)
                        prod = pe.tile([128, H2, DH, M], adt, tag="prod")
                        nc.vector.tensor_tensor(
                            prod[:], v_allT[:, hs],
                            pl_all[:, l, hs, None, :]
                            .to_broadcast([128, H2, DH, M]), AL.mult)
                        half = M // 2
                        nc.vector.tensor_tensor(
                            prod[:, :, :, 0:half], prod[:, :, :, 0:half],
                            prod[:, :, :, half:M], AL.add)
                        nc.vector.tensor_reduce(
                            o_l[:, hs], prod[:, :, :, 0:half],
                            axis=AX.X, op=AL.add)
                    # pooled = o @ wout
                    oTt = pes.tile([128, IT, T], mdt, tag="oTt")
                    for it in range(IT):
                        tps = ptp.tile([128, 128], f32, tag="tp")
                        nc.tensor.transpose(
                            tps[:], o_all[:, l, it * 128:(it + 1) * 128], ident[:])
                        nc.scalar.copy(oTt[:, it], tps[:])
                    pout = pes.tile([128, D], f32, tag="pout")
                    for oc in range(2):
                        ps = pmm.tile([128, 512], f32, tag="mm")
                        for it in range(IT):
                            nc.tensor.matmul(ps[:], oTt[:, it],
                                             wout[:, it, oc * 512:(oc + 1) * 512],
                                             start=(it == 0), stop=(it == IT - 1))
                        nc.scalar.copy(pout[:, oc * 512:(oc + 1) * 512], ps[:])
                    nc.sync.dma_start(out_d[l][:], pout[:])

    nc.compile()
    return nc


def get_nc():
    key = (MM_DT, ATT_DT)
    if key not in _NC_CACHE:
        _NC_CACHE[key] = _build(*key)
    return _NC_CACHE[key]


def prep_weights(w_net, b_net, norm_w, wq, wkv, knorm_w, wg, wout):
    """CPU-side layout prep shared by all cores (fp32 contiguous arrays)."""
    wnetT = np.ascontiguousarray(
        w_net.reshape(L, D, DT, 128).transpose(0, 3, 2, 1))          # [L,128,DT,D]
    colscale = (np.tile(knorm_w, H) * SCALE).astype(np.float32)
    wq2 = norm_w[:, None] * wq * colscale[None, :]
    wqT = np.ascontiguousarray(wq2.reshape(DT, 128, INNER).transpose(1, 0, 2))
    wkvT = np.ascontiguousarray(wkv.reshape(DT, 128, 2 * INNER).transpose(1, 0, 2))
    wgT = np.ascontiguousarray((norm_w[:, None] * wg).reshape(DT, 128, H)
                               .transpose(1, 0, 2))
    woutT = np.ascontiguousarray(wout.reshape(IT, 128, D).transpose(1, 0, 2))
    import ml_dtypes
    bf = ml_dtypes.bfloat16
    return dict(
        wnetT=wnetT.astype(bf),
        wqT=wqT.astype(np.float32),
        wkvT=wkvT.astype(bf),
        wgT=wgT.astype(np.float32),
        woutT=woutT.astype(np.float32),
        bnet=np.ascontiguousarray(b_net[None]).astype(bf),
        onesc=np.ones((128, 128), dtype=np.float32),
        onesb=np.ones((1, 128), dtype=bf),
    )


def prep_core_x(tokens, c):
    """Per-core feature-major token slice: [128, DT, LT]."""
    xs = tokens[:, :, c * NSL:(c + 1) * NSL, :].reshape(L, T, D)
    xT = xs.reshape(L, T, DT, 128).transpose(3, 2, 0, 1).reshape(128, DT, LT)
    return np.ascontiguousarray(xT).astype(np.float32)


def make_in_maps(tokens, w_net, b_net, norm_w, wq, wkv, knorm_w, wg, wout):
    shared = prep_weights(np.asarray(w_net, np.float32), np.asarray(b_net, np.float32),
                          np.asarray(norm_w, np.float32), np.asarray(wq, np.float32),
                          np.asarray(wkv, np.float32), np.asarray(knorm_w, np.float32),
                          np.asarray(wg, np.float32), np.asarray(wout, np.float32))
    import ml_dtypes
    tokens = np.asarray(tokens, np.float32)
    maps = []
    for c in range(NCORES):
        xT = prep_core_x(tokens, c)
        maps.append(dict(shared, xT=xT, xTb=xT.astype(ml_dtypes.bfloat16)))
    return maps


def stitch(results):
    full = np.empty((L, B, N, D), dtype=np.float32)
    for c in range(NCORES):
        full[:, :, c * NSL:(c + 1) * NSL, :] = \
            results[c]["out"].reshape(L, B, NSL, D)
    return full


def kernel(tokens, w_net, b_net, norm_w, wq, wkv, knorm_w, wg, wout):
    nc = get_nc()
    in_maps = make_in_maps(tokens, w_net, b_net, norm_w, wq, wkv, knorm_w, wg, wout)
    res = bass_utils.run_bass_kernel_spmd(nc, in_maps, core_ids=list(range(NCORES)))
    return stitch(res.results)


# revision 15
# speedup vs baseline: 1.7041x; 1.0147x over previous
"""Trainium2 Bass kernel for nn_EnsemblesWithMessagePassing.

Strategy: data-parallel over token positions (shard N=512 across the 8
NeuronCores, 64 positions each => 128 (b,n) tokens per core). The voting
attention is strictly per-position over the M=16 local messages, so this
sharding needs no collectives: each core holds every ensemble member's
weights and computes all 8 members for its position slice.

On-chip dataflow per core (feature-major activations for the PE):
  A) rms stats of tokens via DVE square + ones-column matmul (cross-
     partition sum), rsqrt; a [1,T]-lhsT rank-1 matmul moves the scale to
     token-major [T,1] so it fuses into the q/gates PSUM->SBUF copy.
  B) q = (x @ wq') * rs  and  gates = sigmoid((x @ wg') * rs); norm_w,
     knorm_w and the attention scale are pre-folded into wq'/wg' on CPU.
  C) per member l: kv for the token message; member Linear (token-major,
     bias added via a K=1 rank-1 matmul into the same PSUM accumulation
     group); PE-transpose of the Linear output; kv for the output message.
     k stays RAW -- its rms scale (dh-independent) is folded into sim
     later, so the sim products for a message are emitted immediately
     after its kv and overlap the remaining members' PE work.
  E) batched softmax across all members (sim *= krinv, max-subtracted,
     gate folded into the normalizer), then per member: o = attn*v with
     v stored [h, dh, m]-transposed so the products read unit-stride,
     reduced over m via bf16 pairwise-add levels + a small reduce; then
     PE-transpose of o and pooled = o @ wout.

Precision: stats/q/gates/pooled matmuls in float32r (TF32 datapath,
fp32 PSUM accumulation); member Linear + kv matmuls and attention
elementwise math in bf16 (softmax logits/sums in fp32); everything
normalization-related in fp32.
"""
import sys

for _p in ("/opt/trn_rl_repo", "/root/.axon_site/_ro/trn_rl_repo"):
    if _p not in sys.path:
        sys.path.insert(0, _p)

try:  # NTFF profile hook glue (only needed if BASS_TRACE is set externally)
    import antenv.axon_hooks  # noqa: F401
except Exception:
    try:
        import importlib.util as _ilu
        import antenv as _antenv

        _spec = _ilu.spec_from_file_location(
            "antenv.axon_hooks", "/opt/trn_rl_repo/antenv/axon_hooks.py")
        _mod = _ilu.module_from_spec(_spec)
        _spec.loader.exec_module(_mod)
        _antenv.axon_hooks = _mod
        sys.modules["antenv.axon_hooks"] = _mod
    except Exception:
        pass

from contextlib import ExitStack

import numpy as np

import concourse.bass as bass
import concourse.tile as tile
from concourse import bacc, mybir
from concourse import bass_utils
from concourse.masks import make_identity

f32 = mybir.dt.float32
f32r = mybir.dt.float32r
bf16 = mybir.dt.bfloat16
AF = mybir.ActivationFunctionType
AL = mybir.AluOpType
AX = mybir.AxisListType

# problem shape
L, B, N, D = 8, 2, 512, 1024
H, DH = 8, 64
INNER = H * DH          # 512
M = 2 * L               # 16 messages
SCALE = DH ** -0.5
EPS = float(np.finfo(np.float32).eps)

NCORES = 8
NSL = N // NCORES       # 64 positions per core per batch row
T = B * NSL             # 128 tokens per core
LT = L * T              # 1024
DT = D // 128           # 8 d-tiles
IT = INNER // 128       # 4 inner-tiles

# compute dtype config
MM_DT = "float32r"      # matmul operand dtype: "float32r" | "float32"
ATT_DT = "bfloat16"     # attention DVE dtype: "float32" | "bfloat16"

_NC_CACHE = {}


def _build(mm_dt_name=MM_DT, att_dt_name=ATT_DT):
    mdt = getattr(mybir.dt, mm_dt_name)   # light stages: stats, q, gates, pooled
    adt = getattr(mybir.dt, att_dt_name)  # attention elementwise dtype
    hdt = bf16                            # heavy stages: member Linear + kv

    nc = bacc.Bacc("TRN2", target_bir_lowering=False, debug=False,
                   enable_asserts=False, num_devices=NCORES)

    xT_d = nc.dram_tensor("xT", [128, DT, LT], mdt, kind="ExternalInput").ap()
    xTb_d = nc.dram_tensor("xTb", [128, DT, LT], hdt, kind="ExternalInput").ap()
    wnet_d = nc.dram_tensor("wnetT", [L, 128, DT, D], hdt, kind="ExternalInput").ap()
    wq_d = nc.dram_tensor("wqT", [128, DT, INNER], mdt, kind="ExternalInput").ap()
    wkv_d = nc.dram_tensor("wkvT", [128, DT, 2 * INNER], hdt, kind="ExternalInput").ap()
    wg_d = nc.dram_tensor("wgT", [128, DT, H], mdt, kind="ExternalInput").ap()
    wout_d = nc.dram_tensor("woutT", [128, IT, D], mdt, kind="ExternalInput").ap()
    bnet_d = nc.dram_tensor("bnet", [1, L, D], hdt, kind="ExternalInput").ap()
    onesc_d = nc.dram_tensor("onesc", [128, 128], mdt, kind="ExternalInput").ap()
    onesb_d = nc.dram_tensor("onesb", [1, 128], hdt, kind="ExternalInput").ap()
    out_d = nc.dram_tensor("out", [L, T, D], f32, kind="ExternalOutput").ap()

    def asf32(ap):
        # DVE/ACT read float32r tiles as plain fp32 bits
        return ap.bitcast(f32) if ap.dtype == f32r else ap

    with tile.TileContext(nc) as tc, ExitStack() as ctx:
        pc = ctx.enter_context(tc.tile_pool(name="const", bufs=1))
        pa = ctx.enter_context(tc.tile_pool(name="attp", bufs=1))
        pmm = ctx.enter_context(tc.tile_pool(name="psmm", bufs=4, space="PSUM"))
        ptp = ctx.enter_context(tc.tile_pool(name="pstp", bufs=2, space="PSUM"))
        psm = ctx.enter_context(tc.tile_pool(name="pssm", bufs=2, space="PSUM"))

        # ---- constants ----
        ident = pc.tile([128, 128], f32, tag="ident")
        make_identity(nc, ident)
        ident_b = pc.tile([128, 128], hdt, tag="ident_b")
        make_identity(nc, ident_b)
        ones_m = pc.tile([128, 128], mdt, tag="ones_m")          # f32r ones
        nc.sync.dma_start(ones_m[:], onesc_d[:])
        ones_b = pc.tile([1, 128], hdt, tag="ones_b")            # bf16 ones row
        nc.sync.dma_start(ones_b[:], onesb_d[:])
        ones_f = pc.tile([1, 2], f32, tag="ones_f")              # fp32 ones
        nc.vector.memset(ones_f[:], 1.0)
        eps_c = pc.tile([128, 1], f32, tag="eps")
        nc.vector.memset(eps_c[:], EPS)

        # whole-kernel attention state
        k_all = pa.tile([128, M, INNER], adt, tag="k_all")   # RAW k
        v_allT = pa.tile([128, H, DH, M], adt, tag="v_allT")
        g_all = pa.tile([128, L, H], f32, tag="g_all")
        kss_all = pa.tile([128, M, H], f32, tag="kss_all")
        sim_all = pa.tile([128, L, H, M], f32, tag="sim_all")
        krms = pa.tile([128, M, H], f32, tag="krms")
        krinv = pa.tile([128, M, H], f32, tag="krinv")

        with ExitStack() as ctx_b:
            pb = ctx_b.enter_context(tc.tile_pool(name="bigact", bufs=1))
            ps_ = ctx_b.enter_context(tc.tile_pool(name="scr", bufs=2))

            xT = pb.tile([128, DT, LT], mdt, tag="xT")
            xTb = pb.tile([128, DT, LT], hdt, tag="xTb")
            for d in range(DT):
                nc.sync.dma_start(xT[:, d], xT_d[:, d])
                nc.sync.dma_start(xTb[:, d], xTb_d[:, d])
            wkv = pb.tile([128, DT, 2 * INNER], hdt, tag="wkv")
            nc.sync.dma_start(wkv[:], wkv_d[:])
            q_all = pb.tile([128, L, INNER], adt, tag="q_all")

            with ExitStack() as ctx_w:
                pw = ctx_w.enter_context(tc.tile_pool(name="qgp", bufs=1))
                wq = pw.tile([128, DT, INNER], mdt, tag="wq")
                nc.sync.dma_start(wq[:], wq_d[:])
                wg = pw.tile([128, DT, H], mdt, tag="wg")
                nc.sync.dma_start(wg[:], wg_d[:])

                # ---- phase A: token rms stats ----
                with nc.named_scope("stats"):
                    rms_row = pw.tile([1, LT], f32, tag="rms_row")
                    for c in range(2):
                        ssps = psm.tile([2, 512], f32, tag="sm")
                        for d in range(DT):
                            sq = ps_.tile([128, 512], mdt, tag="sq")
                            nc.vector.tensor_tensor(
                                sq[:], asf32(xT[:, d, c * 512:(c + 1) * 512]),
                                asf32(xT[:, d, c * 512:(c + 1) * 512]), AL.mult)
                            nc.tensor.matmul(ssps[:], ones_m[:, 0:2], sq[:],
                                             start=(d == 0), stop=(d == DT - 1))
                        nc.scalar.activation(rms_row[0:1, c * 512:(c + 1) * 512],
                                             ssps[0:1, :], AF.Sqrt,
                                             scale=1.0 / D, bias=eps_c[0:1, 0:1])
                    rs_row = pw.tile([1, LT], f32, tag="rs_row")
                    nc.vector.reciprocal(rs_row[:], rms_row[:])
                    # move scale to token-major: rs_tok[t, l]
                    rsps = psm.tile([128, L], f32, tag="sm")
                    for l in range(L):
                        nc.tensor.matmul(rsps[:, l:l + 1],
                                         rs_row[0:1, l * T:(l + 1) * T],
                                         ones_f[0:1, 0:1], start=True, stop=True)
                    rs_tok = pw.tile([128, L], f32, tag="rs_tok")
                    nc.vector.tensor_copy(rs_tok[:], rsps[:])

                # ---- phase D: queries and gates ----
                with nc.named_scope("qg"):
                    for l in range(L):
                        qps = pmm.tile([128, INNER], f32, tag="mm")
                        for d in range(DT):
                            nc.tensor.matmul(qps[:], xT[:, d, l * T:(l + 1) * T],
                                             wq[:, d], start=(d == 0),
                                             stop=(d == DT - 1))
                        nc.scalar.activation(q_all[:, l], qps[:], AF.Copy,
                                             scale=rs_tok[:, l:l + 1])
                    for l in range(L):
                        gps = psm.tile([128, H], f32, tag="sm")
                        for d in range(DT):
                            nc.tensor.matmul(gps[:], xT[:, d, l * T:(l + 1) * T],
                                             wg[:, d], start=(d == 0),
                                             stop=(d == DT - 1))
                        nc.scalar.activation(g_all[:, l], gps[:], AF.Sigmoid,
                                             scale=rs_tok[:, l:l + 1])

            # ---- phase C: member Linear + kv + sim products (bf16 stage) ----
            def kv_msg(lhsT_of_dt, m):
                for half in range(2):  # 0 = k, 1 = v
                    ps = pmm.tile([128, INNER], f32, tag="mm")
                    for d in range(DT):
                        nc.tensor.matmul(ps[:], lhsT_of_dt(d),
                                         wkv[:, d, half * INNER:(half + 1) * INNER],
                                         start=(d == 0), stop=(d == DT - 1))
                    if half == 0:
                        nc.scalar.copy(k_all[:, m], ps[:])
                        ksq = ps_.tile([128, INNER], f32, tag="ksq")
                        nc.gpsimd.tensor_tensor(ksq[:], asf32(k_all[:, m]),
                                                asf32(k_all[:, m]), AL.mult)
                        nc.vector.tensor_reduce(
                            kss_all[:, m], ksq.rearrange("p (h d) -> p h d", d=DH),
                            axis=AX.X, op=AL.add)
                    else:
                        nc.scalar.copy(v_allT[:, :, :, m],
                                       ps.rearrange("p (h d) -> p h d", d=DH))

            def sim_msg(m):
                scr8 = ps_.tile([128, L, INNER], adt, tag="scr8")
                for lq in range(L):
                    nc.vector.tensor_tensor(scr8[:, lq], asf32(q_all[:, lq]),
                                            asf32(k_all[:, m]), AL.mult)
                sv = scr8.rearrange("p l (h d) -> p l h d", d=DH)
                nc.vector.tensor_tensor(sv[:, :, :, 0:DH // 2],
                                        sv[:, :, :, 0:DH // 2],
                                        sv[:, :, :, DH // 2:DH], AL.add)
                nc.vector.tensor_tensor(sv[:, :, :, 0:DH // 4],
                                        sv[:, :, :, 0:DH // 4],
                                        sv[:, :, :, DH // 4:DH // 2], AL.add)
                nc.vector.tensor_reduce(
                    sim_all[:, :, :, m], sv[:, :, :, 0:DH // 4],
                    axis=AX.X, op=AL.add)

            with ExitStack() as ctx_l, nc.named_scope("linkv"):
                plk = ctx_l.enter_context(tc.tile_pool(name="linp", bufs=1))
                plw = ctx_l.enter_context(tc.tile_pool(name="linw", bufs=2))
                for l in range(L):
                    kv_msg(lambda d: xTb[:, d, l * T:(l + 1) * T], l)
                    sim_msg(l)
                    # member Linear, token-major out, in 2 chunks of 512 cols
                    bnet_l = plk.tile([1, D], hdt, tag="bnet_l")
                    nc.sync.dma_start(bnet_l[:], bnet_d[0:1, l])
                    osb = plk.tile([128, D], hdt, tag="osb")
                    for oc in range(2):
                        wnt = plw.tile([128, DT, 512], hdt, tag="wnt")
                        nc.sync.dma_start(wnt[:],
                                          wnet_d[l][:, :, oc * 512:(oc + 1) * 512])
                        ps = pmm.tile([128, 512], f32, tag="mm")
                        for d in range(DT):
                            nc.tensor.matmul(ps[:], xTb[:, d, l * T:(l + 1) * T],
                                             wnt[:, d], start=(d == 0), stop=False)
                        nc.tensor.matmul(ps[:], ones_b[0:1, :],
                                         bnet_l[0:1, oc * 512:(oc + 1) * 512],
                                         start=False, stop=True)
                        nc.scalar.copy(osb[:, oc * 512:(oc + 1) * 512], ps[:])
                    # transpose Linear output to feature-major (bf16)
                    oT = plk.tile([128, DT, T], hdt, tag="oT")
                    for ot in range(DT):
                        tps = ptp.tile([128, 128], hdt, tag="tp")
                        nc.tensor.transpose(tps[:], osb[:, ot * 128:(ot + 1) * 128],
                                            ident_b[:])
                        nc.scalar.copy(oT[:, ot], tps[:])
                    kv_msg(lambda d: oT[:, d], L + l)
                    sim_msg(L + l)

            # k rms scales (applied to sim, not k: dh-independent)
            nc.scalar.activation(krms.rearrange("p m h -> p (m h)"),
                                 kss_all.rearrange("p m h -> p (m h)"),
                                 AF.Sqrt, scale=1.0 / DH, bias=eps_c[:, 0:1])
            nc.vector.reciprocal(krinv.rearrange("p m h -> p (m h)"),
                                 krms.rearrange("p m h -> p (m h)"))

        # ---- phase E/F: batched softmax, then o + pooled per member ----
        with ExitStack() as ctx_e:
            pe = ctx_e.enter_context(tc.tile_pool(name="outp", bufs=1))
            pes = ctx_e.enter_context(tc.tile_pool(name="outs", bufs=2))
            wout = pe.tile([128, IT, D], mdt, tag="wout")
            nc.sync.dma_start(wout[:], wout_d[:])
            o_all = pe.tile([128, L, INNER], f32, tag="o_all")
            H2 = H // 2
            with nc.named_scope("attn"):
                # batched softmax over all members at once
                nc.vector.tensor_tensor(
                    sim_all[:], sim_all[:],
                    krinv.rearrange("p m h -> p h m")[:, None]
                    .to_broadcast([128, L, H, M]), AL.mult)
                mx_all = pe.tile([128, L, H], f32, tag="mx_all")
                nc.vector.tensor_reduce(mx_all[:], sim_all[:], axis=AX.X, op=AL.max)
                nc.vector.tensor_tensor(
                    sim_all[:], sim_all[:],
                    mx_all[:, :, :, None].to_broadcast([128, L, H, M]), AL.subtract)
                pl_all = pe.tile([128, L, H, M], adt, tag="pl_all")
                nc.scalar.activation(pl_all.rearrange("p l h m -> p (l h m)"),
                                     sim_all.rearrange("p l h m -> p (l h m)"),
                                     AF.Exp)
                sm_all = pe.tile([128, L, H], f32, tag="sm_all")
                nc.vector.tensor_reduce(sm_all[:], asf32(pl_all[:]),
                                        axis=AX.X, op=AL.add)
                rgf = pe.tile([128, L, H], f32, tag="rgf")
                nc.vector.reciprocal(rgf.rearrange("p l h -> p (l h)"),
                                     sm_all.rearrange("p l h -> p (l h)"))
                rg = pe.tile([128, L, H], adt, tag="rg")
                nc.vector.tensor_tensor(rg[:], rgf[:], g_all[:], AL.mult)
                nc.vector.tensor_tensor(
                    pl_all[:], pl_all[:],
                    rg[:, :, :, None].to_broadcast([128, L, H, M]), AL.mult)
                for l in range(L):
                    o_l = o_all[:, l].rearrange("p (h d) -> p h d", d=DH)
                    for hh in range(2):
                        hs = slice(hh * H2, (hh + 1) * H2)
                        prod = pe.tile([128, H2, DH, M], adt, tag="prod")
                        nc.vector.tensor_tensor(
                            prod[:], v_allT[:, hs],
                            pl_all[:, l, hs, None, :]
                            .to_broadcast([128, H2, DH, M]), AL.mult)
                        half = M // 2
                        quart = M // 4
                        nc.vector.tensor_tensor(
                            prod[:, :, :, 0:half], prod[:, :, :, 0:half],
                            prod[:, :, :, half:M], AL.add)
                        nc.vector.tensor_tensor(
                            prod[:, :, :, 0:quart], prod[:, :, :, 0:quart],
                            prod[:, :, :, quart:half], AL.add)
                        nc.vector.tensor_reduce(
                            o_l[:, hs], prod[:, :, :, 0:quart],
                            axis=AX.X, op=AL.add)
                    # pooled = o @ wout
                    oTt = pes.tile([128, IT, T], mdt, tag="oTt")
                    for it in range(IT):
                        tps = ptp.tile([128, 128], f32, tag="tp")
                        nc.tensor.transpose(
                            tps[:], o_all[:, l, it * 128:(it + 1) * 128], ident[:])
                        nc.scalar.copy(oTt[:, it], tps[:])
                    pout = pes.tile([128, D], f32, tag="pout")
                    for oc in range(2):
                        ps = pmm.tile([128, 512], f32, tag="mm")
                        for it in range(IT):
                            nc.tensor.matmul(ps[:], oTt[:, it],
                                             wout[:, it, oc * 512:(oc + 1) * 512],
                                             start=(it == 0), stop=(it == IT - 1))
                        nc.scalar.copy(pout[:, oc * 512:(oc + 1) * 512], ps[:])
                    nc.sync.dma_start(out_d[l][:], pout[:])

    nc.compile()
    return nc


def get_nc():
    key = (MM_DT, ATT_DT)
    if key not in _NC_CACHE:
        _NC_CACHE[key] = _build(*key)
    return _NC_CACHE[key]


def prep_weights(w_net, b_net, norm_w, wq, wkv, knorm_w, wg, wout):
    """CPU-side layout prep shared by all cores (fp32 contiguous arrays)."""
    wnetT = np.ascontiguousarray(
        w_net.reshape(L, D, DT, 128).transpose(0, 3, 2, 1))          # [L,128,DT,D]
    colscale = (np.tile(knorm_w, H) * SCALE).astype(np.float32)
    wq2 = norm_w[:, None] * wq * colscale[None, :]
    wqT = np.ascontiguousarray(wq2.reshape(DT, 128, INNER).transpose(1, 0, 2))
    wkvT = np.ascontiguousarray(wkv.reshape(DT, 128, 2 * INNER).transpose(1, 0, 2))
    wgT = np.ascontiguousarray((norm_w[:, None] * wg).reshape(DT, 128, H)
                               .transpose(1, 0, 2))
    woutT = np.ascontiguousarray(wout.reshape(IT, 128, D).transpose(1, 0, 2))
    import ml_dtypes
    bf = ml_dtypes.bfloat16
    return dict(
        wnetT=wnetT.astype(bf),
        wqT=wqT.astype(np.float32),
        wkvT=wkvT.astype(bf),
        wgT=wgT.astype(np.float32),
        woutT=woutT.astype(np.float32),
        bnet=np.ascontiguousarray(b_net[None]).astype(bf),
        onesc=np.ones((128, 128), dtype=np.float32),
        onesb=np.ones((1, 128), dtype=bf),
    )


def prep_core_x(tokens, c):
    """Per-core feature-major token slice: [128, DT, LT]."""
    xs = tokens[:, :, c * NSL:(c + 1) * NSL, :].reshape(L, T, D)
    xT = xs.reshape(L, T, DT, 128).transpose(3, 2, 0, 1).reshape(128, DT, LT)
    return np.ascontiguousarray(xT).astype(np.float32)


def make_in_maps(tokens, w_net, b_net, norm_w, wq, wkv, knorm_w, wg, wout):
    shared = prep_weights(np.asarray(w_net, np.float32), np.asarray(b_net, np.float32),
                          np.asarray(norm_w, np.float32), np.asarray(wq, np.float32),
                          np.asarray(wkv, np.float32), np.asarray(knorm_w, np.float32),
                          np.asarray(wg, np.float32), np.asarray(wout, np.float32))
    import ml_dtypes
    tokens = np.asarray(tokens, np.float32)
    maps = []
    for c in range(NCORES):
        xT = prep_core_x(tokens, c)
        maps.append(dict(shared, xT=xT, xTb=xT.astype(ml_dtypes.bfloat16)))
    return maps


def stitch(results):
    full = np.empty((L, B, N, D), dtype=np.float32)
    for c in range(NCORES):
        full[:, :, c * NSL:(c + 1) * NSL, :] = \
            results[c]["out"].reshape(L, B, NSL, D)
    return full


def kernel(tokens, w_net, b_net, norm_w, wq, wkv, knorm_w, wg, wout):
    nc = get_nc()
    in_maps = make_in_maps(tokens, w_net, b_net, norm_w, wq, wkv, knorm_w, wg, wout)
    res = bass_utils.run_bass_kernel_spmd(nc, in_maps, core_ids=list(range(NCORES)))
    return stitch(res.results)
